# revision 17
# baseline (speedup 1.0000x reference)
"""CausalFFTConv on 8 Trainium2 NeuronCores — bf16-IO scan kernel.

y[b,t,d] = sum_{s<=t} x[b,s,d] * k[t-s,d],  k[t,d] = exp(-|decay_d|*t)*cos(freq_d*t)

W-transformed dual-scan algorithm (see kernel_baseline.py): with chunk-local
half-offset phases A(tau)=f*(tau+1/2), c=cos(A), s=sin(A), the post-multiplied
quantities W_C = c*C, W_S = s*S obey first-order ratio recurrences mapping
onto tensor_tensor_scan; y = W_C + W_S runs on the TensorEngine as identity
matmuls into PSUM, staged out (with an fp32->bf16 cast) by the ACT engine.

Resource budget per core (cost model): DMA ~55us of 360 B/ns traffic
(bf16 x in / bf16 y out / fp16 ratio tables), DVE ~57us (all-bf16 premults
in 2x perf mode + fp32 scans + carries), Pool ~58us (scan_C always +
scan_S for POOL_S chunks at 0.833/0.6 ns/elem), ACT ~31us, PE ~29us.
The schedule exists to keep all of those overlapped:

 * head chunks are host-premultiplied (uc,us uploaded directly) so the
   first scans depend only on two small DMAs, not premult tables;
 * two more mid-stream chunks are host-premultiplied to convert spare DMA
   bandwidth into DVE relief;
 * ss2 is derived on-device as 1-cc2 (exact identity) saving a table DMA;
 * rho tables ship in 2 column pieces so short head chunks start early
   (each DMA->consumer edge costs 900ns of semaphore latency);
 * chunk order interleaves batch pairs so one chain's carry latency hides
   under the other chain's scan (batches reset the scan state);
 * short tail chunks shrink the final scan->PE->ACT->DMA drain.

Sharding: d_model (1024) split 8 ways -> 128 channels per core = the
128 SBUF partitions. Full T per core, batch unrolled on the free axis.
"""

import sys

sys.path.insert(0, "/opt/trn_rl_repo")

from contextlib import ExitStack

import ml_dtypes
import numpy as np

import concourse.bass as bass
import concourse.mybir as mybir
from concourse.bass_utils import run_bass_kernel_spmd

B, T, D = 4, 8192, 1024

_RUN_KW: dict = {}
LAST_RESULT = None

NCORES = 8
DP = D // NCORES        # 128 channels per core == SBUF partitions
CH = 2048               # max chunk length == table extent

# per-batch chunk length schedules (sum = T)
_LENS = {
    0: [512, 1024, 1024, 2048, 2048, 1536],
    1: [512, 1024, 1024, 2048, 2048, 1536],
    2: [2048, 2048, 2048, 1536, 512],
    3: [2048, 2048, 2048, 1536, 512],
}
QCOLS = [512, 1024, 1536, 2048]          # carry-constant column per length

# host-premultiplied chunks (uc/us uploaded, no x, no device premult)
HP_SET = {0, 1, 2, 3, 8, 12}
# chunks whose scan_S runs on Pool (the rest run on DVE)
POOL_S = {7, 13, 17, 19}


def _mk_order():
    out = []
    for bp in (0, 2):
        for c in range(len(_LENS[bp])):
            out.append((bp, c))
            out.append((bp + 1, c))
    return out


_ORDER = _mk_order()
NCHUNKS = len(_ORDER)       # 22


class _Chunk:
    __slots__ = ("k", "b", "c", "t0", "L", "first", "last", "qcol", "hp",
                 "spool")

    def __init__(self, k, b, c, t0, L, first, last, qcol, hp, spool):
        self.k, self.b, self.c, self.t0, self.L = k, b, c, t0, L
        self.first, self.last, self.qcol = first, last, qcol
        self.hp, self.spool = hp, spool


def _mk_chunks():
    offs = {b: b * T for b in range(4)}
    chunks = []
    for k, (b, c) in enumerate(_ORDER):
        L = _LENS[b][c]
        t0 = offs[b]
        offs[b] += L
        chunks.append(_Chunk(
            k, b, c, t0, L,
            first=(c == 0), last=(c == len(_LENS[b]) - 1),
            qcol=QCOLS.index(L), hp=(k in HP_SET), spool=(k in POOL_S),
        ))
    return chunks


CHUNKS = _mk_chunks()
FIRST_NONHP = min(ch.k for ch in CHUNKS if not ch.hp)
# first chunk needing full-width (>1024) tables on each scan path
FIRST_WIDE = min(ch.k for ch in CHUNKS if ch.L > 1024)
FIRST_WIDE_SD = min(ch.k for ch in CHUNKS if ch.L > 1024 and not ch.spool)
# cumulative counters (value of the sem once chunk k's item is done)
PREM_IDX = np.cumsum([0 if ch.hp else 1 for ch in CHUNKS]).tolist()
HP_IDX = np.cumsum([1 if ch.hp else 0 for ch in CHUNKS]).tolist()
POOL_S_IDX = np.cumsum([1 if ch.spool else 0 for ch in CHUNKS]).tolist()
DVE_S_IDX = np.cumsum([0 if ch.spool else 1 for ch in CHUNKS]).tolist()
CARRY_CNT = np.cumsum([0 if ch.last else 1 for ch in CHUNKS]).tolist()

_F32 = mybir.dt.float32
_F32R = mybir.dt.float32r
_F16 = mybir.dt.float16
_BF16 = mybir.dt.bfloat16
_MUL = mybir.AluOpType.mult
_ADD = mybir.AluOpType.add

NXSLOT = 6      # x / uc / us chunk slots
NWSLOT = 3      # wc / ws scan-output slots
NYSLOT = 4      # y staging slots
NQ = len(QCOLS)


def _build_nc():
    nc = bass.Bass()
    xs_len = sum(ch.L for ch in CHUNKS if not ch.hp)
    up_len = sum(2 * ch.L for ch in CHUNKS if ch.hp)
    xs = nc.declare_dram_parameter("xs", [DP, xs_len], _BF16, isOutput=False)
    ups = nc.declare_dram_parameter("ups", [DP, up_len], _BF16, isOutput=False)
    cc2 = nc.declare_dram_parameter("cc2", [DP, CH], _BF16, isOutput=False)
    rhoC = nc.declare_dram_parameter("rhoC", [DP, CH], _F16, isOutput=False)
    rhoS = nc.declare_dram_parameter("rhoS", [DP, CH], _F16, isOutput=False)
    # fused carry constants, one column pair per chunk length:
    #   Winit_C = qc2[:,2q]*WcEnd + qs2[:,2q]*WsEnd
    #   Winit_S = qc2[:,2q+1]*WcEnd + qs2[:,2q+1]*WsEnd
    qc2 = nc.declare_dram_parameter("qc2", [DP, 2 * NQ], _F32, isOutput=False)
    qs2 = nc.declare_dram_parameter("qs2", [DP, 2 * NQ], _F32, isOutput=False)
    ident = nc.declare_dram_parameter("ident", [DP, DP], _F32R, isOutput=False)
    ys = nc.declare_dram_parameter("ys", [DP, B * T], _BF16, isOutput=True)

    # per-chunk offsets into xs / ups
    xoff = {}
    uoff = {}
    xo = uo = 0
    for ch in CHUNKS:
        if ch.hp:
            uoff[ch.k] = uo
            uo += 2 * ch.L
        else:
            xoff[ch.k] = xo
            xo += ch.L

    with ExitStack() as ctx:
        ent = ctx.enter_context
        cc2_sb = ent(nc.sbuf_tensor([DP, CH], _BF16))
        ss2_sb = ent(nc.sbuf_tensor([DP, CH], _BF16))
        rhoC_sb = ent(nc.sbuf_tensor([DP, CH], _F16))
        rhoS_sb = ent(nc.sbuf_tensor([DP, CH], _F16))
        qc2_sb = ent(nc.sbuf_tensor([DP, 2 * NQ], _F32))
        qs2_sb = ent(nc.sbuf_tensor([DP, 2 * NQ], _F32))
        xt_sb = ent(nc.sbuf_tensor([DP, NXSLOT * CH], _BF16))
        uc_sb = ent(nc.sbuf_tensor([DP, NXSLOT * CH], _BF16))
        us_sb = ent(nc.sbuf_tensor([DP, NXSLOT * CH], _BF16))
        id_sb = ent(nc.sbuf_tensor([DP, DP], _F32R))
        y_sb = ent(nc.sbuf_tensor([DP, NYSLOT * CH], _BF16))
        wc_sb = ent(nc.sbuf_tensor([DP, NWSLOT * CH], _F32R))
        ws_sb = ent(nc.sbuf_tensor([DP, NWSLOT * CH], _F32R))
        iq_sb = ent(nc.sbuf_tensor([DP, 2 * 4], _F32))   # per-batch inits
        t0_sb = ent(nc.sbuf_tensor([DP, 2], _F32))       # carry scratch
        y_ps = ent(nc.psum_tensor([DP, 2 * CH], _F32))
        dma_in = ent(nc.semaphore("dma_in"))
        dma_hp = ent(nc.semaphore("dma_hp"))
        dma_tab = ent(nc.semaphore("dma_tab"))
        dma_out = ent(nc.semaphore("dma_out"))
        premC = ent(nc.semaphore("premC"))
        premS = ent(nc.semaphore("premS"))
        scC = ent(nc.semaphore("scC"))
        scSd = ent(nc.semaphore("scSd"))
        scSp = ent(nc.semaphore("scSp"))
        carry = ent(nc.semaphore("carry"))
        pe_y = ent(nc.semaphore("pe_y"))
        act_y = ent(nc.semaphore("act_y"))
        block = ent(nc.Block(no_gpsimd_drain=True))

        # table DMA sem values, recorded while emitting the sync program
        tabv = {}

        def _s_wait(eng, k):
            """wait until chunk k's scan_S is complete"""
            if CHUNKS[k].spool:
                eng.wait_ge(scSp, POOL_S_IDX[k])
            else:
                eng.wait_ge(scSd, DVE_S_IDX[k])

        @block.sync
        def _(sync: bass.BassEngine):
            ntab = [0]

            def tab(name, out, in_):
                sync.dma_start(out=out, in_=in_).then_inc(dma_tab, 16)
                ntab[0] += 16
                tabv[name] = ntab[0]

            def xdma(k):
                ch = CHUNKS[k]
                i = k % NXSLOT
                if not ch.hp:
                    j = k - NXSLOT
                    if j >= 0:
                        # WAR: slot consumers of chunk j must be done.  The
                        # xt slot is read by premults; uc/us by the scans.
                        if CHUNKS[j].hp:
                            sync.wait_ge(scC, j + 1)
                            _s_wait(sync, j)
                        else:
                            sync.wait_ge(premC, PREM_IDX[j])
                            sync.wait_ge(premS, PREM_IDX[j])
                    sync.dma_start(
                        out=xt_sb[:, i * CH:i * CH + ch.L],
                        in_=xs[:, xoff[k]:xoff[k] + ch.L],
                    ).then_inc(dma_in, 16)
                else:
                    j = k - NXSLOT
                    if j >= 0:
                        # WAR on uc/us slots: scans of j must be done
                        sync.wait_ge(scC, j + 1)
                        _s_wait(sync, j)
                    uo = uoff[k]
                    sync.dma_start(
                        out=uc_sb[:, i * CH:i * CH + ch.L],
                        in_=ups[:, uo:uo + ch.L],
                    ).then_inc(dma_hp, 16)
                    sync.dma_start(
                        out=us_sb[:, i * CH:i * CH + ch.L],
                        in_=ups[:, uo + ch.L:uo + 2 * ch.L],
                    ).then_inc(dma_hp, 16)

            # head: hp chunks 0..3 interleaved with table pieces
            xdma(0)
            tab("rCp", rhoC_sb[:, :1024], rhoC[:, :1024])
            tab("rSp", rhoS_sb[:, :1024], rhoS[:, :1024])
            tab("id", id_sb[:], ident[:])
            tab("q", qc2_sb[:], qc2[:])
            tab("q2", qs2_sb[:], qs2[:])
            xdma(1)
            xdma(2)
            xdma(3)
            tab("cc2", cc2_sb[:], cc2[:])
            xdma(4)
            xdma(5)
            tab("rCf", rhoC_sb[:, 1024:], rhoC[:, 1024:])
            tab("rSf", rhoS_sb[:, 1024:], rhoS[:, 1024:])
            for k in range(6, NCHUNKS):
                xdma(k)
            sync.wait_ge(dma_out, NCHUNKS * 16)

        @block.vector
        def _(vector: bass.BassEngine):
            def prem(k):
                if k >= NCHUNKS or CHUNKS[k].hp:
                    return
                ch = CHUNKS[k]
                i = k % NXSLOT
                xt = xt_sb[:, i * CH:i * CH + ch.L]
                vector.wait_ge(dma_in, PREM_IDX[k] * 16)
                if k == FIRST_NONHP:
                    vector.wait_ge(dma_tab, tabv["cc2"])
                j = k - NXSLOT
                if j >= 0:
                    # WAR on uc slot: scan_C of j (on Pool) read it
                    vector.wait_ge(scC, j + 1)
                vector.tensor_tensor(
                    out=uc_sb[:, i * CH:i * CH + ch.L], in0=xt,
                    in1=cc2_sb[:, :ch.L], op=_MUL,
                ).then_inc(premC, 1)
                if k == FIRST_NONHP:
                    # ss2 = 1 - cc2 (exact identity), bf16 4x tensor_scalar
                    vector.tensor_scalar(
                        out=ss2_sb[:], in0=cc2_sb[:], scalar1=-1.0,
                        scalar2=1.0, op0=_MUL, op1=_ADD,
                    )
                if j >= 0 and CHUNKS[j].spool:
                    # WAR on us slot (DVE-run scan_S is ordered by our queue)
                    vector.wait_ge(scSp, POOL_S_IDX[j])
                vector.tensor_tensor(
                    out=us_sb[:, i * CH:i * CH + ch.L], in0=xt,
                    in1=ss2_sb[:, :ch.L], op=_MUL,
                ).then_inc(premS, 1)

            prem(0)
            for k in range(NCHUNKS):
                ch = CHUNKS[k]
                b = ch.b
                j = k % NWSLOT
                i = k % NXSLOT

                if not ch.spool:
                    # scan_S on DVE
                    if k == 0:
                        vector.wait_ge(dma_tab, tabv["rSp"])
                    elif k == FIRST_WIDE_SD:
                        vector.wait_ge(dma_tab, tabv["rSf"])
                    if ch.hp:
                        vector.wait_ge(dma_hp, HP_IDX[k] * 32)
                    if k >= 2:
                        vector.wait_ge(carry, CARRY_CNT[k - 2])
                    if k >= NWSLOT:
                        vector.wait_ge(pe_y, k - NWSLOT + 1)   # WAR ws slot
                    init_s = 0.0 if ch.first else iq_sb[:, 2 * b + 1:2 * b + 2]
                    vector.tensor_tensor_scan(
                        out=ws_sb[:, j * CH:j * CH + ch.L],
                        data0=rhoS_sb[:, :ch.L],
                        data1=us_sb[:, i * CH:i * CH + ch.L],
                        initial=init_s, op0=_MUL, op1=_ADD,
                    ).then_inc(scSd, 1)

                if not ch.last:
                    # fused carry for chunk (b, c+1):
                    #   t0 = [qcc,qsc]*WcEnd ; iq[2b:2b+2] = [qcs,qss]*WsEnd+t0
                    if k == 0:
                        vector.wait_ge(dma_tab, tabv["q2"])
                    vector.wait_ge(scC, k + 1)
                    if ch.spool:
                        vector.wait_ge(scSp, POOL_S_IDX[k])
                    q = ch.qcol
                    wce = wc_sb[:, j * CH + ch.L - 1:j * CH + ch.L].bitcast(_F32)
                    wse = ws_sb[:, j * CH + ch.L - 1:j * CH + ch.L].bitcast(_F32)
                    vector.tensor_scalar_mul(
                        out=t0_sb[:], in0=qc2_sb[:, 2 * q:2 * q + 2],
                        scalar1=wce,
                    )
                    vector.scalar_tensor_tensor(
                        out=iq_sb[:, 2 * b:2 * b + 2],
                        in0=qs2_sb[:, 2 * q:2 * q + 2],
                        scalar=wse, in1=t0_sb[:], op0=_MUL, op1=_ADD,
                    ).then_inc(carry, 1)

                prem(k + 1)

        @block.gpsimd
        def _(gpsimd: bass.BassEngine):
            for k in range(NCHUNKS):
                ch = CHUNKS[k]
                b = ch.b
                j = k % NWSLOT
                i = k % NXSLOT
                if k == 0:
                    gpsimd.wait_ge(dma_tab, tabv["rCp"])
                elif k == FIRST_WIDE:
                    gpsimd.wait_ge(dma_tab, tabv["rCf"])
                if ch.hp:
                    gpsimd.wait_ge(dma_hp, HP_IDX[k] * 32 - 16)
                else:
                    gpsimd.wait_ge(premC, PREM_IDX[k])
                if k >= 2:
                    gpsimd.wait_ge(carry, CARRY_CNT[k - 2])
                if k >= NWSLOT:
                    gpsimd.wait_ge(pe_y, k - NWSLOT + 1)    # WAR wc slot
                init_c = 0.0 if ch.first else iq_sb[:, 2 * b:2 * b + 1]
                gpsimd.tensor_tensor_scan(
                    out=wc_sb[:, j * CH:j * CH + ch.L],
                    data0=rhoC_sb[:, :ch.L],
                    data1=uc_sb[:, i * CH:i * CH + ch.L],
                    initial=init_c, op0=_MUL, op1=_ADD,
                ).then_inc(scC, 1)
                if ch.spool:
                    if POOL_S_IDX[k] == 1:
                        gpsimd.wait_ge(dma_tab, tabv["rSf"])
                    if ch.hp:
                        gpsimd.wait_ge(dma_hp, HP_IDX[k] * 32)
                    else:
                        gpsimd.wait_ge(premS, PREM_IDX[k])
                    init_s = 0.0 if ch.first else iq_sb[:, 2 * b + 1:2 * b + 2]
                    gpsimd.tensor_tensor_scan(
                        out=ws_sb[:, j * CH:j * CH + ch.L],
                        data0=rhoS_sb[:, :ch.L],
                        data1=us_sb[:, i * CH:i * CH + ch.L],
                        initial=init_s, op0=_MUL, op1=_ADD,
                    ).then_inc(scSp, 1)

        @block.tensor
        def _(tensor: bass.BassEngine):
            tensor.wait_ge(dma_tab, tabv["id"])
            for k in range(NCHUNKS):
                ch = CHUNKS[k]
                i2 = k % 2
                j = k % NWSLOT
                tensor.wait_ge(scC, k + 1)
                _s_wait(tensor, k)
                if k >= 2:
                    # WAR: ACT copy of k-2 must have drained this PSUM half
                    tensor.wait_ge(act_y, k - 1)
                nseg = (ch.L + 511) // 512
                mm = None
                for seg in range(nseg):
                    sl = min(512, ch.L - seg * 512)
                    pb = i2 * CH + seg * 512
                    wb = j * CH + seg * 512
                    tensor.matmul(
                        y_ps[:, pb:pb + sl],
                        id_sb[:],
                        wc_sb[:, wb:wb + sl],
                        start=True, stop=False,
                    )
                    mm = tensor.matmul(
                        y_ps[:, pb:pb + sl],
                        id_sb[:],
                        ws_sb[:, wb:wb + sl],
                        start=False, stop=True,
                    )
                mm.then_inc(pe_y, 1)

        @block.scalar
        def _(scalar: bass.BassEngine):
            for k in range(NCHUNKS):
                ch = CHUNKS[k]
                i2 = k % 2
                i4 = k % NYSLOT
                scalar.wait_ge(pe_y, k + 1)
                if k >= NYSLOT:
                    # WAR on y_sb slot: out-DMA of k-NYSLOT must have drained
                    scalar.wait_ge(dma_out, (k - NYSLOT + 1) * 16)
                scalar.copy(
                    out=y_sb[:, i4 * CH:i4 * CH + ch.L],
                    in_=y_ps[:, i2 * CH:i2 * CH + ch.L],
                ).then_inc(act_y, 1)
                # dma_start is a SEQ-level trigger: without this wait it
                # races the still-executing copy on the ACT engine pipe
                scalar.wait_ge(act_y, k + 1)
                scalar.dma_start(
                    out=ys[:, ch.t0:ch.t0 + ch.L],
                    in_=y_sb[:, i4 * CH:i4 * CH + ch.L],
                ).then_inc(dma_out, 16)

    return nc


def _host_tables(decay: np.ndarray, freq: np.ndarray):
    """float64 table construction, cast to fp32/fp16/bf16 at the end."""
    a = np.abs(decay.astype(np.float64))
    f = freq.astype(np.float64)
    damp = np.exp(-a)

    tau = np.arange(CH, dtype=np.float64) + 0.5
    A = f[:, None] * tau[None, :]         # [D, CH]
    c = np.cos(A)
    s = np.sin(A)
    # clamp |cos|, |sin| away from zero so the fp16 ratio tables stay in
    # range (max ratio ~ 1/eps = 125 << fp16 max); the induced kernel error
    # is O(eps^2) at isolated taus.
    eps = 8e-3
    c = np.where(np.abs(c) < eps, np.where(c >= 0, eps, -eps), c)
    s = np.where(np.abs(s) < eps, np.where(s >= 0, eps, -eps), s)
    # weight at tau = -1/2 (the scan-initial position)
    w0c = np.cos(-0.5 * f)
    w0s = np.sin(-0.5 * f)
    w0c = np.where(np.abs(w0c) < eps, eps, w0c)
    w0s = np.where(np.abs(w0s) < eps, np.where(w0s >= 0, eps, -eps), w0s)

    rhoC = np.empty_like(c)
    rhoS = np.empty_like(s)
    rhoC[:, 0] = damp * c[:, 0] / w0c
    rhoS[:, 0] = damp * s[:, 0] / w0s
    rhoC[:, 1:] = damp[:, None] * c[:, 1:] / c[:, :-1]
    rhoS[:, 1:] = damp[:, None] * s[:, 1:] / s[:, :-1]

    # carry across a boundary after a chunk of length L:
    #   g' = e^{+i f L} g, g = C - iS =>
    #   C' = cos(fL) C + sin(fL) S ;  S' = cos(fL) S - sin(fL) C
    #   C_end = Wc_end / c[L-1], S_end = Ws_end / s[L-1]
    #   Winit_C = w0c * C', Winit_S = w0s * S'
    # columns interleaved as [qcc,qsc] (qc2) / [qcs,qss] (qs2) per length
    qc2 = np.empty((len(f), 2 * NQ))
    qs2 = np.empty_like(qc2)
    for qi, L in enumerate(QCOLS):
        rc = np.cos(f * L)
        rs = np.sin(f * L)
        qc2[:, 2 * qi] = w0c * rc / c[:, L - 1]        # qcc
        qc2[:, 2 * qi + 1] = -w0s * rs / c[:, L - 1]   # qsc
        qs2[:, 2 * qi] = w0c * rs / s[:, L - 1]        # qcs
        qs2[:, 2 * qi + 1] = w0s * rc / s[:, L - 1]    # qss

    f32 = np.float32
    return (
        (c * c).astype(ml_dtypes.bfloat16),
        rhoC.astype(np.float16), rhoS.astype(np.float16),
        qc2.astype(f32), qs2.astype(f32),
        c * c, s * s,           # float64 copies for host premult
    )


def kernel(x: np.ndarray, decay: np.ndarray, freq: np.ndarray) -> np.ndarray:
    x = np.asarray(x)
    decay = np.asarray(decay)
    freq = np.asarray(freq)
    assert x.shape == (B, T, D), x.shape
    cc2, rhoC, rhoS, qc2, qs2, cc2_64, ss2_64 = _host_tables(decay, freq)

    # [B,T,D] -> [D, B*T] contiguous, split by core
    xf = np.ascontiguousarray(x.transpose(2, 0, 1).reshape(D, B * T))

    # pack xs (non-hp chunks) and ups (host-premultiplied uc,us pairs)
    bf16 = ml_dtypes.bfloat16
    xs_parts = []
    up_parts = []
    for ch in CHUNKS:
        seg = xf[:, ch.t0:ch.t0 + ch.L]
        if ch.hp:
            up_parts.append((seg * cc2_64[:, :ch.L]).astype(bf16))
            up_parts.append((seg * ss2_64[:, :ch.L]).astype(bf16))
        else:
            xs_parts.append(seg.astype(bf16))
    xs = np.concatenate(xs_parts, axis=1)
    ups = np.concatenate(up_parts, axis=1) if up_parts else \
        np.zeros((D, 0), bf16)

    in_maps = []
    for cidx in range(NCORES):
        lo, hi = cidx * DP, (cidx + 1) * DP
        in_maps.append(
            {
                "xs": np.ascontiguousarray(xs[lo:hi]),
                "ups": np.ascontiguousarray(ups[lo:hi]),
                "cc2": cc2[lo:hi],
                "rhoC": np.ascontiguousarray(rhoC[lo:hi]),
                "rhoS": np.ascontiguousarray(rhoS[lo:hi]),
                "qc2": np.ascontiguousarray(qc2[lo:hi]),
                "qs2": np.ascontiguousarray(qs2[lo:hi]),
                "ident": np.eye(DP, dtype=np.float32),
            }
        )

    nc = _build_nc()
    res = run_bass_kernel_spmd(nc, in_maps, list(range(NCORES)), **_RUN_KW)

    global LAST_RESULT
    LAST_RESULT = res
    y = np.empty((D, B * T), np.float32)
    for cidx in range(NCORES):
        y[cidx * DP:(cidx + 1) * DP] = np.asarray(
            res.results[cidx]["ys"]
        ).astype(np.float32)
    return np.ascontiguousarray(
        y.reshape(D, B, T).transpose(1, 2, 0)
    ).astype(x.dtype)


if __name__ == "__main__":
    rng = np.random.default_rng(0)
    x = rng.standard_normal((B, T, D)).astype(np.float32)
    decay = rng.standard_normal(D).astype(np.float32)
    freq = rng.standard_normal(D).astype(np.float32)
    y = kernel(x, decay, freq)
    print(y.shape, y.dtype, np.abs(y).mean())


# revision 24
# speedup vs baseline: 1.0480x; 1.0480x over previous
"""CausalFFTConv on 8 Trainium2 NeuronCores — bf16-IO scan kernel.

y[b,t,d] = sum_{s<=t} x[b,s,d] * k[t-s,d],  k[t,d] = exp(-|decay_d|*t)*cos(freq_d*t)

W-transformed dual-scan algorithm (see kernel_baseline.py): with chunk-local
half-offset phases A(tau)=f*(tau+1/2), c=cos(A), s=sin(A), the post-multiplied
quantities W_C = c*C, W_S = s*S obey first-order ratio recurrences mapping
onto tensor_tensor_scan; y = W_C + W_S runs on the TensorEngine as identity
matmuls into PSUM, staged out (with an fp32->bf16 cast) by the ACT engine.

Resource budget per core (cost model): DMA ~55us of 360 B/ns traffic
(bf16 x in / bf16 y out / fp16 ratio tables), DVE ~57us (all-bf16 premults
in 2x perf mode + fp32 scans + carries), Pool ~58us (scan_C always +
scan_S for POOL_S chunks at 0.833/0.6 ns/elem), ACT ~31us, PE ~29us.
The schedule exists to keep all of those overlapped:

 * head chunks are host-premultiplied (uc,us uploaded directly) so the
   first scans depend only on two small DMAs, not premult tables;
 * two more mid-stream chunks are host-premultiplied to convert spare DMA
   bandwidth into DVE relief;
 * ss2 is derived on-device as 1-cc2 (exact identity) saving a table DMA;
 * rho tables ship in 2 column pieces so short head chunks start early
   (each DMA->consumer edge costs 900ns of semaphore latency);
 * chunk order interleaves batch pairs so one chain's carry latency hides
   under the other chain's scan (batches reset the scan state);
 * short tail chunks shrink the final scan->PE->ACT->DMA drain.

Sharding: d_model (1024) split 8 ways -> 128 channels per core = the
128 SBUF partitions. Full T per core, batch unrolled on the free axis.
"""

import sys

sys.path.insert(0, "/opt/trn_rl_repo")

from contextlib import ExitStack

import ml_dtypes
import numpy as np

import concourse.bass as bass
import concourse.mybir as mybir
from concourse.bass_utils import run_bass_kernel_spmd

B, T, D = 4, 8192, 1024

_RUN_KW: dict = {}
LAST_RESULT = None

NCORES = 8
DP = D // NCORES        # 128 channels per core == SBUF partitions
CH = 2048               # max chunk length == table extent

# per-batch chunk length schedules (sum = T)
_LENS = {
    0: [512, 1024, 1024, 2048, 2048, 1536],
    1: [512, 1024, 1024, 2048, 2048, 1536],
    2: [2048, 2048, 2048, 1536, 512],
    3: [2048, 2048, 2048, 1536, 512],
}
QCOLS = [512, 1024, 1536, 2048]          # carry-constant column per length

# host-premultiplied chunks (uc/us uploaded, no x, no device premult)
HP_SET = {0, 1, 2, 3, 8, 12}
# chunks whose scan_S runs on Pool (the rest run on DVE)
POOL_S = {7, 13, 17, 19}


def _mk_order():
    out = []
    for bp in (0, 2):
        for c in range(len(_LENS[bp])):
            out.append((bp, c))
            out.append((bp + 1, c))
    return out


_ORDER = _mk_order()
NCHUNKS = len(_ORDER)       # 22


class _Chunk:
    __slots__ = ("k", "b", "c", "t0", "L", "first", "last", "qcol", "hp",
                 "spool")

    def __init__(self, k, b, c, t0, L, first, last, qcol, hp, spool):
        self.k, self.b, self.c, self.t0, self.L = k, b, c, t0, L
        self.first, self.last, self.qcol = first, last, qcol
        self.hp, self.spool = hp, spool


def _mk_chunks():
    offs = {b: b * T for b in range(4)}
    chunks = []
    for k, (b, c) in enumerate(_ORDER):
        L = _LENS[b][c]
        t0 = offs[b]
        offs[b] += L
        chunks.append(_Chunk(
            k, b, c, t0, L,
            first=(c == 0), last=(c == len(_LENS[b]) - 1),
            qcol=QCOLS.index(L), hp=(k in HP_SET), spool=(k in POOL_S),
        ))
    return chunks


CHUNKS = _mk_chunks()
FIRST_NONHP = min(ch.k for ch in CHUNKS if not ch.hp)
# first chunk needing full-width (>1024) tables on each scan path
FIRST_WIDE = min(ch.k for ch in CHUNKS if ch.L > 1024)
FIRST_WIDE_SD = min(ch.k for ch in CHUNKS if ch.L > 1024 and not ch.spool)
# cumulative counters (value of the sem once chunk k's item is done)
PREM_IDX = np.cumsum([0 if ch.hp else 1 for ch in CHUNKS]).tolist()
HP_IDX = np.cumsum([1 if ch.hp else 0 for ch in CHUNKS]).tolist()
POOL_S_IDX = np.cumsum([1 if ch.spool else 0 for ch in CHUNKS]).tolist()
DVE_S_IDX = np.cumsum([0 if ch.spool else 1 for ch in CHUNKS]).tolist()
# carries run on the engine that produced the chunk's scan_S: Pool for
# POOL_S chunks, DVE otherwise.  Separate counters per engine.
CARRY_D_IDX = np.cumsum(
    [1 if (not ch.last and not ch.spool) else 0 for ch in CHUNKS]).tolist()
CARRY_P_IDX = np.cumsum(
    [1 if (not ch.last and ch.spool) else 0 for ch in CHUNKS]).tolist()

_F32 = mybir.dt.float32
_F32R = mybir.dt.float32r
_F16 = mybir.dt.float16
_BF16 = mybir.dt.bfloat16
_MUL = mybir.AluOpType.mult
_ADD = mybir.AluOpType.add

NXSLOT = 6      # x / uc / us chunk slots
NWSLOT = 3      # wc / ws scan-output slots
NYSLOT = 4      # y staging slots
NQ = len(QCOLS)


def _build_nc():
    nc = bass.Bass()
    xs_len = sum(ch.L for ch in CHUNKS if not ch.hp)
    up_len = sum(2 * ch.L for ch in CHUNKS if ch.hp)
    xs = nc.declare_dram_parameter("xs", [DP, xs_len], _BF16, isOutput=False)
    ups = nc.declare_dram_parameter("ups", [DP, up_len], _BF16, isOutput=False)
    cc2 = nc.declare_dram_parameter("cc2", [DP, CH], _BF16, isOutput=False)
    rhoC = nc.declare_dram_parameter("rhoC", [DP, CH], _F16, isOutput=False)
    rhoS = nc.declare_dram_parameter("rhoS", [DP, CH], _F16, isOutput=False)
    # fused carry constants, one column pair per chunk length:
    #   Winit_C = qc2[:,2q]*WcEnd + qs2[:,2q]*WsEnd
    #   Winit_S = qc2[:,2q+1]*WcEnd + qs2[:,2q+1]*WsEnd
    qc2 = nc.declare_dram_parameter("qc2", [DP, 2 * NQ], _F32, isOutput=False)
    qs2 = nc.declare_dram_parameter("qs2", [DP, 2 * NQ], _F32, isOutput=False)
    ident = nc.declare_dram_parameter("ident", [DP, DP], _F32R, isOutput=False)
    ys = nc.declare_dram_parameter("ys", [DP, B * T], _BF16, isOutput=True)

    # per-chunk offsets into xs / ups
    xoff = {}
    uoff = {}
    xo = uo = 0
    for ch in CHUNKS:
        if ch.hp:
            uoff[ch.k] = uo
            uo += 2 * ch.L
        else:
            xoff[ch.k] = xo
            xo += ch.L

    with ExitStack() as ctx:
        ent = ctx.enter_context
        cc2_sb = ent(nc.sbuf_tensor([DP, CH], _BF16))
        ss2_sb = ent(nc.sbuf_tensor([DP, CH], _BF16))
        rhoC_sb = ent(nc.sbuf_tensor([DP, CH], _F16))
        rhoS_sb = ent(nc.sbuf_tensor([DP, CH], _F16))
        qc2_sb = ent(nc.sbuf_tensor([DP, 2 * NQ], _F32))
        qs2_sb = ent(nc.sbuf_tensor([DP, 2 * NQ], _F32))
        xt_sb = ent(nc.sbuf_tensor([DP, NXSLOT * CH], _BF16))
        uc_sb = ent(nc.sbuf_tensor([DP, NXSLOT * CH], _BF16))
        us_sb = ent(nc.sbuf_tensor([DP, NXSLOT * CH], _BF16))
        id_sb = ent(nc.sbuf_tensor([DP, DP], _F32R))
        y_sb = ent(nc.sbuf_tensor([DP, NYSLOT * CH], _BF16))
        wc_sb = ent(nc.sbuf_tensor([DP, NWSLOT * CH], _F32R))
        ws_sb = ent(nc.sbuf_tensor([DP, NWSLOT * CH], _F32R))
        iq_sb = ent(nc.sbuf_tensor([DP, 2 * 4], _F32))   # per-batch inits
        t0_sb = ent(nc.sbuf_tensor([DP, 2], _F32))       # DVE carry scratch
        t0p_sb = ent(nc.sbuf_tensor([DP, 2], _F32))      # Pool carry scratch
        y_ps = ent(nc.psum_tensor([DP, 2 * CH], _F32))
        dma_in = ent(nc.semaphore("dma_in"))
        dma_hp = ent(nc.semaphore("dma_hp"))
        dma_tab = ent(nc.semaphore("dma_tab"))
        dma_out = ent(nc.semaphore("dma_out"))
        premC = ent(nc.semaphore("premC"))
        premS = ent(nc.semaphore("premS"))
        scC = ent(nc.semaphore("scC"))
        scSd = ent(nc.semaphore("scSd"))
        scSp = ent(nc.semaphore("scSp"))
        carryD = ent(nc.semaphore("carryD"))
        carryP = ent(nc.semaphore("carryP"))
        pe_y = ent(nc.semaphore("pe_y"))
        act_y = ent(nc.semaphore("act_y"))
        block = ent(nc.Block(no_gpsimd_drain=True))

        # table DMA sem values, recorded while emitting the sync program
        tabv = {}

        def _s_wait(eng, k):
            """wait until chunk k's scan_S is complete"""
            if CHUNKS[k].spool:
                eng.wait_ge(scSp, POOL_S_IDX[k])
            else:
                eng.wait_ge(scSd, DVE_S_IDX[k])

        def _carry_wait(eng, j, own):
            """wait until chunk j's carry is complete (own = this engine
            runs that carry itself, so program order covers it)"""
            if j < 0 or CHUNKS[j].last:
                return
            if CHUNKS[j].spool:
                if own != "pool":
                    eng.wait_ge(carryP, CARRY_P_IDX[j])
            else:
                if own != "dve":
                    eng.wait_ge(carryD, CARRY_D_IDX[j])

        @block.sync
        def _(sync: bass.BassEngine):
            ntab = [0]

            def tab(name, out, in_):
                sync.dma_start(out=out, in_=in_).then_inc(dma_tab, 16)
                ntab[0] += 16
                tabv[name] = ntab[0]

            def xdma(k):
                ch = CHUNKS[k]
                i = k % NXSLOT
                if not ch.hp:
                    j = k - NXSLOT
                    if j >= 0:
                        # WAR: slot consumers of chunk j must be done.  The
                        # xt slot is read by premults; uc/us by the scans.
                        if CHUNKS[j].hp:
                            sync.wait_ge(scC, j + 1)
                            _s_wait(sync, j)
                        else:
                            sync.wait_ge(premC, PREM_IDX[j])
                            sync.wait_ge(premS, PREM_IDX[j])
                    sync.dma_start(
                        out=xt_sb[:, i * CH:i * CH + ch.L],
                        in_=xs[:, xoff[k]:xoff[k] + ch.L],
                    ).then_inc(dma_in, 16)
                else:
                    j = k - NXSLOT
                    if j >= 0:
                        # WAR on uc/us slots: scans of j must be done
                        sync.wait_ge(scC, j + 1)
                        _s_wait(sync, j)
                    uo = uoff[k]
                    sync.dma_start(
                        out=uc_sb[:, i * CH:i * CH + ch.L],
                        in_=ups[:, uo:uo + ch.L],
                    ).then_inc(dma_hp, 16)
                    sync.dma_start(
                        out=us_sb[:, i * CH:i * CH + ch.L],
                        in_=ups[:, uo + ch.L:uo + 2 * ch.L],
                    ).then_inc(dma_hp, 16)

            # head: hp chunks 0..3 interleaved with table pieces
            xdma(0)
            tab("rCp", rhoC_sb[:, :1024], rhoC[:, :1024])
            tab("rSp", rhoS_sb[:, :1024], rhoS[:, :1024])
            tab("id", id_sb[:], ident[:])
            tab("q", qc2_sb[:], qc2[:])
            tab("q2", qs2_sb[:], qs2[:])
            xdma(1)
            xdma(2)
            xdma(3)
            tab("cc2", cc2_sb[:], cc2[:])
            xdma(4)
            xdma(5)
            tab("rCf", rhoC_sb[:, 1024:], rhoC[:, 1024:])
            tab("rSf", rhoS_sb[:, 1024:], rhoS[:, 1024:])
            for k in range(6, NCHUNKS):
                xdma(k)
            sync.wait_ge(dma_out, NCHUNKS * 16)

        @block.vector
        def _(vector: bass.BassEngine):
            def prem(k):
                if k >= NCHUNKS or CHUNKS[k].hp:
                    return
                ch = CHUNKS[k]
                i = k % NXSLOT
                xt = xt_sb[:, i * CH:i * CH + ch.L]
                vector.wait_ge(dma_in, PREM_IDX[k] * 16)
                if k == FIRST_NONHP:
                    vector.wait_ge(dma_tab, tabv["cc2"])
                j = k - NXSLOT
                if j >= 0:
                    # WAR on uc slot: scan_C of j (on Pool) read it
                    vector.wait_ge(scC, j + 1)
                vector.tensor_tensor(
                    out=uc_sb[:, i * CH:i * CH + ch.L], in0=xt,
                    in1=cc2_sb[:, :ch.L], op=_MUL,
                ).then_inc(premC, 1)
                if k == FIRST_NONHP:
                    # ss2 = 1 - cc2 (exact identity), bf16 4x tensor_scalar
                    vector.tensor_scalar(
                        out=ss2_sb[:], in0=cc2_sb[:], scalar1=-1.0,
                        scalar2=1.0, op0=_MUL, op1=_ADD,
                    )
                if j >= 0 and CHUNKS[j].spool:
                    # WAR on us slot (DVE-run scan_S is ordered by our queue)
                    vector.wait_ge(scSp, POOL_S_IDX[j])
                vector.tensor_tensor(
                    out=us_sb[:, i * CH:i * CH + ch.L], in0=xt,
                    in1=ss2_sb[:, :ch.L], op=_MUL,
                ).then_inc(premS, 1)

            prem(0)
            for k in range(NCHUNKS):
                ch = CHUNKS[k]
                b = ch.b
                j = k % NWSLOT
                i = k % NXSLOT

                if not ch.spool:
                    # scan_S on DVE
                    if k == 0:
                        vector.wait_ge(dma_tab, tabv["rSp"])
                    elif k == FIRST_WIDE_SD:
                        vector.wait_ge(dma_tab, tabv["rSf"])
                    if ch.hp:
                        vector.wait_ge(dma_hp, HP_IDX[k] * 32)
                    _carry_wait(vector, k - 2, "dve")          # chain init
                    _carry_wait(vector, k - 3, "dve")          # WAR ws end
                    if k >= NWSLOT:
                        vector.wait_ge(pe_y, k - NWSLOT + 1)   # WAR ws slot
                    init_s = 0.0 if ch.first else iq_sb[:, 2 * b + 1:2 * b + 2]
                    vector.tensor_tensor_scan(
                        out=ws_sb[:, j * CH:j * CH + ch.L],
                        data0=rhoS_sb[:, :ch.L],
                        data1=us_sb[:, i * CH:i * CH + ch.L],
                        initial=init_s, op0=_MUL, op1=_ADD,
                    ).then_inc(scSd, 1)

                if not ch.last and not ch.spool:
                    # fused carry for chunk (b, c+1):
                    #   t0 = [qcc,qsc]*WcEnd ; iq[2b:2b+2] = [qcs,qss]*WsEnd+t0
                    if k == 0:
                        vector.wait_ge(dma_tab, tabv["q2"])
                    vector.wait_ge(scC, k + 1)
                    q = ch.qcol
                    wce = wc_sb[:, j * CH + ch.L - 1:j * CH + ch.L].bitcast(_F32)
                    wse = ws_sb[:, j * CH + ch.L - 1:j * CH + ch.L].bitcast(_F32)
                    vector.tensor_scalar_mul(
                        out=t0_sb[:], in0=qc2_sb[:, 2 * q:2 * q + 2],
                        scalar1=wce,
                    )
                    vector.scalar_tensor_tensor(
                        out=iq_sb[:, 2 * b:2 * b + 2],
                        in0=qs2_sb[:, 2 * q:2 * q + 2],
                        scalar=wse, in1=t0_sb[:], op0=_MUL, op1=_ADD,
                    ).then_inc(carryD, 1)

                prem(k + 1)

        @block.gpsimd
        def _(gpsimd: bass.BassEngine):
            for k in range(NCHUNKS):
                ch = CHUNKS[k]
                b = ch.b
                j = k % NWSLOT
                i = k % NXSLOT
                if k == 0:
                    gpsimd.wait_ge(dma_tab, tabv["rCp"])
                elif k == FIRST_WIDE:
                    gpsimd.wait_ge(dma_tab, tabv["rCf"])
                if ch.hp:
                    gpsimd.wait_ge(dma_hp, HP_IDX[k] * 32 - 16)
                else:
                    gpsimd.wait_ge(premC, PREM_IDX[k])
                _carry_wait(gpsimd, k - 2, "pool")          # chain init
                _carry_wait(gpsimd, k - 3, "pool")          # WAR wc end
                if k >= NWSLOT:
                    gpsimd.wait_ge(pe_y, k - NWSLOT + 1)    # WAR wc slot
                init_c = 0.0 if ch.first else iq_sb[:, 2 * b:2 * b + 1]
                gpsimd.tensor_tensor_scan(
                    out=wc_sb[:, j * CH:j * CH + ch.L],
                    data0=rhoC_sb[:, :ch.L],
                    data1=uc_sb[:, i * CH:i * CH + ch.L],
                    initial=init_c, op0=_MUL, op1=_ADD,
                ).then_inc(scC, 1)
                if ch.spool:
                    if POOL_S_IDX[k] == 1:
                        gpsimd.wait_ge(dma_tab, tabv["rSf"])
                        gpsimd.wait_ge(dma_tab, tabv["q2"])
                    if ch.hp:
                        gpsimd.wait_ge(dma_hp, HP_IDX[k] * 32)
                    else:
                        gpsimd.wait_ge(premS, PREM_IDX[k])
                    init_s = 0.0 if ch.first else iq_sb[:, 2 * b + 1:2 * b + 2]
                    gpsimd.tensor_tensor_scan(
                        out=ws_sb[:, j * CH:j * CH + ch.L],
                        data0=rhoS_sb[:, :ch.L],
                        data1=us_sb[:, i * CH:i * CH + ch.L],
                        initial=init_s, op0=_MUL, op1=_ADD,
                    ).then_inc(scSp, 1)
                    if not ch.last:
                        # carry on Pool: both scan ends are local here
                        q = ch.qcol
                        wce = wc_sb[:, j * CH + ch.L - 1:j * CH + ch.L]\
                            .bitcast(_F32)
                        wse = ws_sb[:, j * CH + ch.L - 1:j * CH + ch.L]\
                            .bitcast(_F32)
                        gpsimd.tensor_scalar_mul(
                            out=t0p_sb[:], in0=qc2_sb[:, 2 * q:2 * q + 2],
                            scalar1=wce,
                        )
                        gpsimd.scalar_tensor_tensor(
                            out=iq_sb[:, 2 * b:2 * b + 2],
                            in0=qs2_sb[:, 2 * q:2 * q + 2],
                            scalar=wse, in1=t0p_sb[:], op0=_MUL, op1=_ADD,
                        ).then_inc(carryP, 1)

        @block.tensor
        def _(tensor: bass.BassEngine):
            tensor.wait_ge(dma_tab, tabv["id"])
            for k in range(NCHUNKS):
                ch = CHUNKS[k]
                i2 = k % 2
                j = k % NWSLOT
                tensor.wait_ge(scC, k + 1)
                _s_wait(tensor, k)
                if k >= 2:
                    # WAR: ACT copy of k-2 must have drained this PSUM half
                    tensor.wait_ge(act_y, k - 1)
                nseg = (ch.L + 511) // 512
                mm = None
                for seg in range(nseg):
                    sl = min(512, ch.L - seg * 512)
                    pb = i2 * CH + seg * 512
                    wb = j * CH + seg * 512
                    tensor.matmul(
                        y_ps[:, pb:pb + sl],
                        id_sb[:],
                        wc_sb[:, wb:wb + sl],
                        start=True, stop=False,
                    )
                    mm = tensor.matmul(
                        y_ps[:, pb:pb + sl],
                        id_sb[:],
                        ws_sb[:, wb:wb + sl],
                        start=False, stop=True,
                    )
                mm.then_inc(pe_y, 1)

        @block.scalar
        def _(scalar: bass.BassEngine):
            for k in range(NCHUNKS):
                ch = CHUNKS[k]
                i2 = k % 2
                i4 = k % NYSLOT
                scalar.wait_ge(pe_y, k + 1)
                if k >= NYSLOT:
                    # WAR on y_sb slot: out-DMA of k-NYSLOT must have drained
                    scalar.wait_ge(dma_out, (k - NYSLOT + 1) * 16)
                scalar.copy(
                    out=y_sb[:, i4 * CH:i4 * CH + ch.L],
                    in_=y_ps[:, i2 * CH:i2 * CH + ch.L],
                ).then_inc(act_y, 1)
                # dma_start is a SEQ-level trigger: without this wait it
                # races the still-executing copy on the ACT engine pipe
                scalar.wait_ge(act_y, k + 1)
                scalar.dma_start(
                    out=ys[:, ch.t0:ch.t0 + ch.L],
                    in_=y_sb[:, i4 * CH:i4 * CH + ch.L],
                ).then_inc(dma_out, 16)

    return nc


def _host_tables(decay: np.ndarray, freq: np.ndarray):
    """float64 table construction, cast to fp32/fp16/bf16 at the end."""
    a = np.abs(decay.astype(np.float64))
    f = freq.astype(np.float64)
    damp = np.exp(-a)

    tau = np.arange(CH, dtype=np.float64) + 0.5
    A = f[:, None] * tau[None, :]         # [D, CH]
    c = np.cos(A)
    s = np.sin(A)
    # clamp |cos|, |sin| away from zero so the fp16 ratio tables stay in
    # range (max ratio ~ 1/eps = 125 << fp16 max); the induced kernel error
    # is O(eps^2) at isolated taus.
    eps = 8e-3
    c = np.where(np.abs(c) < eps, np.where(c >= 0, eps, -eps), c)
    s = np.where(np.abs(s) < eps, np.where(s >= 0, eps, -eps), s)
    # weight at tau = -1/2 (the scan-initial position)
    w0c = np.cos(-0.5 * f)
    w0s = np.sin(-0.5 * f)
    w0c = np.where(np.abs(w0c) < eps, eps, w0c)
    w0s = np.where(np.abs(w0s) < eps, np.where(w0s >= 0, eps, -eps), w0s)

    rhoC = np.empty_like(c)
    rhoS = np.empty_like(s)
    rhoC[:, 0] = damp * c[:, 0] / w0c
    rhoS[:, 0] = damp * s[:, 0] / w0s
    rhoC[:, 1:] = damp[:, None] * c[:, 1:] / c[:, :-1]
    rhoS[:, 1:] = damp[:, None] * s[:, 1:] / s[:, :-1]

    # carry across a boundary after a chunk of length L:
    #   g' = e^{+i f L} g, g = C - iS =>
    #   C' = cos(fL) C + sin(fL) S ;  S' = cos(fL) S - sin(fL) C
    #   C_end = Wc_end / c[L-1], S_end = Ws_end / s[L-1]
    #   Winit_C = w0c * C', Winit_S = w0s * S'
    # columns interleaved as [qcc,qsc] (qc2) / [qcs,qss] (qs2) per length
    qc2 = np.empty((len(f), 2 * NQ))
    qs2 = np.empty_like(qc2)
    for qi, L in enumerate(QCOLS):
        rc = np.cos(f * L)
        rs = np.sin(f * L)
        qc2[:, 2 * qi] = w0c * rc / c[:, L - 1]        # qcc
        qc2[:, 2 * qi + 1] = -w0s * rs / c[:, L - 1]   # qsc
        qs2[:, 2 * qi] = w0c * rs / s[:, L - 1]        # qcs
        qs2[:, 2 * qi + 1] = w0s * rc / s[:, L - 1]    # qss

    f32 = np.float32
    return (
        (c * c).astype(ml_dtypes.bfloat16),
        rhoC.astype(np.float16), rhoS.astype(np.float16),
        qc2.astype(f32), qs2.astype(f32),
        c * c, s * s,           # float64 copies for host premult
    )


def kernel(x: np.ndarray, decay: np.ndarray, freq: np.ndarray) -> np.ndarray:
    x = np.asarray(x)
    decay = np.asarray(decay)
    freq = np.asarray(freq)
    assert x.shape == (B, T, D), x.shape
    cc2, rhoC, rhoS, qc2, qs2, cc2_64, ss2_64 = _host_tables(decay, freq)

    # [B,T,D] -> [D, B*T] contiguous, split by core
    xf = np.ascontiguousarray(x.transpose(2, 0, 1).reshape(D, B * T))

    # pack xs (non-hp chunks) and ups (host-premultiplied uc,us pairs)
    bf16 = ml_dtypes.bfloat16
    xs_parts = []
    up_parts = []
    for ch in CHUNKS:
        seg = xf[:, ch.t0:ch.t0 + ch.L]
        if ch.hp:
            up_parts.append((seg * cc2_64[:, :ch.L]).astype(bf16))
            up_parts.append((seg * ss2_64[:, :ch.L]).astype(bf16))
        else:
            xs_parts.append(seg.astype(bf16))
    xs = np.concatenate(xs_parts, axis=1)
    ups = np.concatenate(up_parts, axis=1) if up_parts else \
        np.zeros((D, 0), bf16)

    in_maps = []
    for cidx in range(NCORES):
        lo, hi = cidx * DP, (cidx + 1) * DP
        in_maps.append(
            {
                "xs": np.ascontiguousarray(xs[lo:hi]),
                "ups": np.ascontiguousarray(ups[lo:hi]),
                "cc2": cc2[lo:hi],
                "rhoC": np.ascontiguousarray(rhoC[lo:hi]),
                "rhoS": np.ascontiguousarray(rhoS[lo:hi]),
                "qc2": np.ascontiguousarray(qc2[lo:hi]),
                "qs2": np.ascontiguousarray(qs2[lo:hi]),
                "ident": np.eye(DP, dtype=np.float32),
            }
        )

    nc = _build_nc()
    res = run_bass_kernel_spmd(nc, in_maps, list(range(NCORES)), **_RUN_KW)

    global LAST_RESULT
    LAST_RESULT = res
    y = np.empty((D, B * T), np.float32)
    for cidx in range(NCORES):
        y[cidx * DP:(cidx + 1) * DP] = np.asarray(
            res.results[cidx]["ys"]
        ).astype(np.float32)
    return np.ascontiguousarray(
        y.reshape(D, B, T).transpose(1, 2, 0)
    ).astype(x.dtype)


if __name__ == "__main__":
    rng = np.random.default_rng(0)
    x = rng.standard_normal((B, T, D)).astype(np.float32)
    decay = rng.standard_normal(D).astype(np.float32)
    freq = rng.standard_normal(D).astype(np.float32)
    y = kernel(x, decay, freq)
    print(y.shape, y.dtype, np.abs(y).mean())


# revision 27
# speedup vs baseline: 1.2224x; 1.1663x over previous
"""CausalFFTConv on 8 Trainium2 NeuronCores — bf16-IO scan kernel.

y[b,t,d] = sum_{s<=t} x[b,s,d] * k[t-s,d],  k[t,d] = exp(-|decay_d|*t)*cos(freq_d*t)

W-transformed dual-scan algorithm (see kernel_baseline.py): with chunk-local
half-offset phases A(tau)=f*(tau+1/2), c=cos(A), s=sin(A), the post-multiplied
quantities W_C = c*C, W_S = s*S obey first-order ratio recurrences mapping
onto tensor_tensor_scan; y = W_C + W_S runs on the TensorEngine as identity
matmuls into PSUM, staged out (with an fp32->bf16 cast) by the ACT engine.

Resource budget per core (cost model): DMA ~55us of 360 B/ns traffic
(bf16 x in / bf16 y out / fp16 ratio tables), DVE ~57us (all-bf16 premults
in 2x perf mode + fp32 scans + carries), Pool ~58us (scan_C always +
scan_S for POOL_S chunks at 0.833/0.6 ns/elem), ACT ~31us, PE ~29us.
The schedule exists to keep all of those overlapped:

 * head chunks are host-premultiplied (uc,us uploaded directly) so the
   first scans depend only on two small DMAs, not premult tables;
 * two more mid-stream chunks are host-premultiplied to convert spare DMA
   bandwidth into DVE relief;
 * ss2 is derived on-device as 1-cc2 (exact identity) saving a table DMA;
 * rho tables ship in 2 column pieces so short head chunks start early
   (each DMA->consumer edge costs 900ns of semaphore latency);
 * chunk order interleaves batch pairs so one chain's carry latency hides
   under the other chain's scan (batches reset the scan state);
 * short tail chunks shrink the final scan->PE->ACT->DMA drain.

Sharding: d_model (1024) split 8 ways -> 128 channels per core = the
128 SBUF partitions. Full T per core, batch unrolled on the free axis.
"""

import sys

sys.path.insert(0, "/opt/trn_rl_repo")

from contextlib import ExitStack

import ml_dtypes
import numpy as np

import concourse.bass as bass
import concourse.mybir as mybir
from concourse.bass_utils import run_bass_kernel_spmd

B, T, D = 4, 8192, 1024

_RUN_KW: dict = {}
LAST_RESULT = None

NCORES = 8
DP = D // NCORES        # 128 channels per core == SBUF partitions
CH = 2048               # max chunk length == table extent

# per-batch chunk length schedules (sum = T)
_LENS = {
    0: [512, 1024, 1024, 2048, 2048, 1536],
    1: [512, 1024, 1024, 2048, 2048, 1536],
    2: [2048, 2048, 2048, 1536, 512],
    3: [2048, 2048, 2048, 1536, 512],
}
QCOLS = [512, 1024, 1536, 2048]          # carry-constant column per length

# host-premultiplied chunks (uc/us uploaded, no x, no device premult)
HP_SET = {0, 1, 2, 3, 8, 12}
# chunks whose scan_S runs on Pool (the rest run on DVE)
POOL_S = {7, 13, 17, 19}


def _mk_order():
    out = []
    for bp in (0, 2):
        for c in range(len(_LENS[bp])):
            out.append((bp, c))
            out.append((bp + 1, c))
    return out


_ORDER = _mk_order()
NCHUNKS = len(_ORDER)       # 22


class _Chunk:
    __slots__ = ("k", "b", "c", "t0", "L", "first", "last", "qcol", "hp",
                 "spool")

    def __init__(self, k, b, c, t0, L, first, last, qcol, hp, spool):
        self.k, self.b, self.c, self.t0, self.L = k, b, c, t0, L
        self.first, self.last, self.qcol = first, last, qcol
        self.hp, self.spool = hp, spool


def _mk_chunks():
    offs = {b: b * T for b in range(4)}
    chunks = []
    for k, (b, c) in enumerate(_ORDER):
        L = _LENS[b][c]
        t0 = offs[b]
        offs[b] += L
        chunks.append(_Chunk(
            k, b, c, t0, L,
            first=(c == 0), last=(c == len(_LENS[b]) - 1),
            qcol=QCOLS.index(L), hp=(k in HP_SET), spool=(k in POOL_S),
        ))
    return chunks


CHUNKS = _mk_chunks()
FIRST_NONHP = min(ch.k for ch in CHUNKS if not ch.hp)
# first chunk needing full-width (>1024) tables on each scan path
FIRST_WIDE = min(ch.k for ch in CHUNKS if ch.L > 1024)
FIRST_WIDE_SD = min(ch.k for ch in CHUNKS if ch.L > 1024 and not ch.spool)
# cumulative counters (value of the sem once chunk k's item is done)
PREM_IDX = np.cumsum([0 if ch.hp else 1 for ch in CHUNKS]).tolist()
HP_IDX = np.cumsum([1 if ch.hp else 0 for ch in CHUNKS]).tolist()
POOL_S_IDX = np.cumsum([1 if ch.spool else 0 for ch in CHUNKS]).tolist()
DVE_S_IDX = np.cumsum([0 if ch.spool else 1 for ch in CHUNKS]).tolist()
# carries run on the engine that produced the chunk's scan_S: Pool for
# POOL_S chunks, DVE otherwise.  Separate counters per engine.
CARRY_D_IDX = np.cumsum(
    [1 if (not ch.last and not ch.spool) else 0 for ch in CHUNKS]).tolist()
CARRY_P_IDX = np.cumsum(
    [1 if (not ch.last and ch.spool) else 0 for ch in CHUNKS]).tolist()

_F32 = mybir.dt.float32
_F32R = mybir.dt.float32r
_F16 = mybir.dt.float16
_BF16 = mybir.dt.bfloat16
_MUL = mybir.AluOpType.mult
_ADD = mybir.AluOpType.add

NXSLOT = 6      # x / uc / us chunk slots
NWSLOT = 4      # wc / ws scan-output slots
NYSLOT = 4      # y staging slots
PREM_AHEAD = 2  # premult emission lookahead (chunks)
NQ = len(QCOLS)


def _build_nc():
    nc = bass.Bass()
    xs_len = sum(ch.L for ch in CHUNKS if not ch.hp)
    up_len = sum(2 * ch.L for ch in CHUNKS if ch.hp)
    xs = nc.declare_dram_parameter("xs", [DP, xs_len], _BF16, isOutput=False)
    ups = nc.declare_dram_parameter("ups", [DP, up_len], _BF16, isOutput=False)
    cc2 = nc.declare_dram_parameter("cc2", [DP, CH], _BF16, isOutput=False)
    rhoC = nc.declare_dram_parameter("rhoC", [DP, CH], _F16, isOutput=False)
    rhoS = nc.declare_dram_parameter("rhoS", [DP, CH], _F16, isOutput=False)
    # fused carry constants, one column pair per chunk length:
    #   Winit_C = qc2[:,2q]*WcEnd + qs2[:,2q]*WsEnd
    #   Winit_S = qc2[:,2q+1]*WcEnd + qs2[:,2q+1]*WsEnd
    qc2 = nc.declare_dram_parameter("qc2", [DP, 2 * NQ], _F32, isOutput=False)
    qs2 = nc.declare_dram_parameter("qs2", [DP, 2 * NQ], _F32, isOutput=False)
    ident = nc.declare_dram_parameter("ident", [DP, DP], _F32R, isOutput=False)
    ys = nc.declare_dram_parameter("ys", [DP, B * T], _BF16, isOutput=True)

    # per-chunk offsets into xs / ups
    xoff = {}
    uoff = {}
    xo = uo = 0
    for ch in CHUNKS:
        if ch.hp:
            uoff[ch.k] = uo
            uo += 2 * ch.L
        else:
            xoff[ch.k] = xo
            xo += ch.L

    with ExitStack() as ctx:
        ent = ctx.enter_context
        cc2_sb = ent(nc.sbuf_tensor([DP, CH], _BF16))
        ss2_sb = ent(nc.sbuf_tensor([DP, CH], _BF16))
        rhoC_sb = ent(nc.sbuf_tensor([DP, CH], _F16))
        rhoS_sb = ent(nc.sbuf_tensor([DP, CH], _F16))
        qc2_sb = ent(nc.sbuf_tensor([DP, 2 * NQ], _F32))
        qs2_sb = ent(nc.sbuf_tensor([DP, 2 * NQ], _F32))
        xt_sb = ent(nc.sbuf_tensor([DP, NXSLOT * CH], _BF16))
        uc_sb = ent(nc.sbuf_tensor([DP, NXSLOT * CH], _BF16))
        us_sb = ent(nc.sbuf_tensor([DP, NXSLOT * CH], _BF16))
        id_sb = ent(nc.sbuf_tensor([DP, DP], _F32R))
        y_sb = ent(nc.sbuf_tensor([DP, NYSLOT * CH], _BF16))
        wc_sb = ent(nc.sbuf_tensor([DP, NWSLOT * CH], _F32R))
        ws_sb = ent(nc.sbuf_tensor([DP, NWSLOT * CH], _F32R))
        iq_sb = ent(nc.sbuf_tensor([DP, 2 * 4], _F32))   # per-batch inits
        t0_sb = ent(nc.sbuf_tensor([DP, 2], _F32))       # DVE carry scratch
        t0p_sb = ent(nc.sbuf_tensor([DP, 2], _F32))      # Pool carry scratch
        y_ps = ent(nc.psum_tensor([DP, 2 * CH], _F32))
        dma_in = ent(nc.semaphore("dma_in"))
        dma_hp = ent(nc.semaphore("dma_hp"))
        dma_tab = ent(nc.semaphore("dma_tab"))
        dma_out = ent(nc.semaphore("dma_out"))
        premC = ent(nc.semaphore("premC"))
        premS = ent(nc.semaphore("premS"))
        scC = ent(nc.semaphore("scC"))
        scSd = ent(nc.semaphore("scSd"))
        scSp = ent(nc.semaphore("scSp"))
        carryD = ent(nc.semaphore("carryD"))
        carryP = ent(nc.semaphore("carryP"))
        pe_y = ent(nc.semaphore("pe_y"))
        act_y = ent(nc.semaphore("act_y"))
        block = ent(nc.Block(no_gpsimd_drain=True))

        # table DMA sem values, recorded while emitting the sync program
        tabv = {}

        def _s_wait(eng, k):
            """wait until chunk k's scan_S is complete"""
            if CHUNKS[k].spool:
                eng.wait_ge(scSp, POOL_S_IDX[k])
            else:
                eng.wait_ge(scSd, DVE_S_IDX[k])

        def _carry_wait(eng, j, own):
            """wait until chunk j's carry is complete (own = this engine
            runs that carry itself, so program order covers it)"""
            if j < 0 or CHUNKS[j].last:
                return
            if CHUNKS[j].spool:
                if own != "pool":
                    eng.wait_ge(carryP, CARRY_P_IDX[j])
            else:
                if own != "dve":
                    eng.wait_ge(carryD, CARRY_D_IDX[j])

        @block.sync
        def _(sync: bass.BassEngine):
            ntab = [0]

            def tab(name, out, in_):
                sync.dma_start(out=out, in_=in_).then_inc(dma_tab, 16)
                ntab[0] += 16
                tabv[name] = ntab[0]

            def xdma(k):
                ch = CHUNKS[k]
                i = k % NXSLOT
                if not ch.hp:
                    j = k - NXSLOT
                    if j >= 0:
                        # WAR: slot consumers of chunk j must be done.  The
                        # xt slot is read by premults; uc/us by the scans.
                        if CHUNKS[j].hp:
                            sync.wait_ge(scC, j + 1)
                            _s_wait(sync, j)
                        else:
                            sync.wait_ge(premC, PREM_IDX[j])
                            sync.wait_ge(premS, PREM_IDX[j])
                    sync.dma_start(
                        out=xt_sb[:, i * CH:i * CH + ch.L],
                        in_=xs[:, xoff[k]:xoff[k] + ch.L],
                    ).then_inc(dma_in, 16)
                else:
                    j = k - NXSLOT
                    if j >= 0:
                        # WAR on uc/us slots: scans of j must be done
                        sync.wait_ge(scC, j + 1)
                        _s_wait(sync, j)
                    uo = uoff[k]
                    sync.dma_start(
                        out=uc_sb[:, i * CH:i * CH + ch.L],
                        in_=ups[:, uo:uo + ch.L],
                    ).then_inc(dma_hp, 16)
                    sync.dma_start(
                        out=us_sb[:, i * CH:i * CH + ch.L],
                        in_=ups[:, uo + ch.L:uo + 2 * ch.L],
                    ).then_inc(dma_hp, 16)

            # head: hp chunks 0..3 interleaved with table pieces
            xdma(0)
            tab("rCp", rhoC_sb[:, :1024], rhoC[:, :1024])
            tab("rSp", rhoS_sb[:, :1024], rhoS[:, :1024])
            tab("id", id_sb[:], ident[:])
            tab("q", qc2_sb[:], qc2[:])
            tab("q2", qs2_sb[:], qs2[:])
            xdma(1)
            xdma(2)
            xdma(3)
            tab("cc2", cc2_sb[:], cc2[:])
            xdma(4)
            xdma(5)
            tab("rCf", rhoC_sb[:, 1024:], rhoC[:, 1024:])
            tab("rSf", rhoS_sb[:, 1024:], rhoS[:, 1024:])
            for k in range(6, NCHUNKS):
                xdma(k)
            sync.wait_ge(dma_out, NCHUNKS * 16)

        @block.vector
        def _(vector: bass.BassEngine):
            def prem(k):
                if k >= NCHUNKS or CHUNKS[k].hp:
                    return
                ch = CHUNKS[k]
                i = k % NXSLOT
                xt = xt_sb[:, i * CH:i * CH + ch.L]
                vector.wait_ge(dma_in, PREM_IDX[k] * 16)
                if k == FIRST_NONHP:
                    vector.wait_ge(dma_tab, tabv["cc2"])
                j = k - NXSLOT
                if j >= 0:
                    # WAR on uc slot: scan_C of j (on Pool) read it
                    vector.wait_ge(scC, j + 1)
                vector.tensor_tensor(
                    out=uc_sb[:, i * CH:i * CH + ch.L], in0=xt,
                    in1=cc2_sb[:, :ch.L], op=_MUL,
                ).then_inc(premC, 1)
                if k == FIRST_NONHP:
                    # ss2 = 1 - cc2 (exact identity), bf16 4x tensor_scalar
                    vector.tensor_scalar(
                        out=ss2_sb[:], in0=cc2_sb[:], scalar1=-1.0,
                        scalar2=1.0, op0=_MUL, op1=_ADD,
                    )
                if j >= 0 and CHUNKS[j].spool:
                    # WAR on us slot (DVE-run scan_S is ordered by our queue)
                    vector.wait_ge(scSp, POOL_S_IDX[j])
                vector.tensor_tensor(
                    out=us_sb[:, i * CH:i * CH + ch.L], in0=xt,
                    in1=ss2_sb[:, :ch.L], op=_MUL,
                ).then_inc(premS, 1)

            for kk in range(PREM_AHEAD):
                prem(kk)
            for k in range(NCHUNKS):
                ch = CHUNKS[k]
                b = ch.b
                j = k % NWSLOT
                i = k % NXSLOT

                if not ch.spool:
                    # scan_S on DVE
                    if k == 0:
                        vector.wait_ge(dma_tab, tabv["rSp"])
                    elif k == FIRST_WIDE_SD:
                        vector.wait_ge(dma_tab, tabv["rSf"])
                    if ch.hp:
                        vector.wait_ge(dma_hp, HP_IDX[k] * 32)
                    _carry_wait(vector, k - 2, "dve")          # chain init
                    _carry_wait(vector, k - 3, "dve")          # WAR ws end
                    if k >= NWSLOT:
                        vector.wait_ge(pe_y, k - NWSLOT + 1)   # WAR ws slot
                    init_s = 0.0 if ch.first else iq_sb[:, 2 * b + 1:2 * b + 2]
                    vector.tensor_tensor_scan(
                        out=ws_sb[:, j * CH:j * CH + ch.L],
                        data0=rhoS_sb[:, :ch.L],
                        data1=us_sb[:, i * CH:i * CH + ch.L],
                        initial=init_s, op0=_MUL, op1=_ADD,
                    ).then_inc(scSd, 1)

                if not ch.last and not ch.spool:
                    # fused carry for chunk (b, c+1):
                    #   t0 = [qcc,qsc]*WcEnd ; iq[2b:2b+2] = [qcs,qss]*WsEnd+t0
                    if k == 0:
                        vector.wait_ge(dma_tab, tabv["q2"])
                    vector.wait_ge(scC, k + 1)
                    q = ch.qcol
                    wce = wc_sb[:, j * CH + ch.L - 1:j * CH + ch.L].bitcast(_F32)
                    wse = ws_sb[:, j * CH + ch.L - 1:j * CH + ch.L].bitcast(_F32)
                    vector.tensor_scalar_mul(
                        out=t0_sb[:], in0=qc2_sb[:, 2 * q:2 * q + 2],
                        scalar1=wce,
                    )
                    vector.scalar_tensor_tensor(
                        out=iq_sb[:, 2 * b:2 * b + 2],
                        in0=qs2_sb[:, 2 * q:2 * q + 2],
                        scalar=wse, in1=t0_sb[:], op0=_MUL, op1=_ADD,
                    ).then_inc(carryD, 1)

                prem(k + PREM_AHEAD)

        @block.gpsimd
        def _(gpsimd: bass.BassEngine):
            for k in range(NCHUNKS):
                ch = CHUNKS[k]
                b = ch.b
                j = k % NWSLOT
                i = k % NXSLOT
                if k == 0:
                    gpsimd.wait_ge(dma_tab, tabv["rCp"])
                elif k == FIRST_WIDE:
                    gpsimd.wait_ge(dma_tab, tabv["rCf"])
                if ch.hp:
                    gpsimd.wait_ge(dma_hp, HP_IDX[k] * 32 - 16)
                else:
                    gpsimd.wait_ge(premC, PREM_IDX[k])
                _carry_wait(gpsimd, k - 2, "pool")          # chain init
                _carry_wait(gpsimd, k - 3, "pool")          # WAR wc end
                if k >= NWSLOT:
                    gpsimd.wait_ge(pe_y, k - NWSLOT + 1)    # WAR wc slot
                init_c = 0.0 if ch.first else iq_sb[:, 2 * b:2 * b + 1]
                gpsimd.tensor_tensor_scan(
                    out=wc_sb[:, j * CH:j * CH + ch.L],
                    data0=rhoC_sb[:, :ch.L],
                    data1=uc_sb[:, i * CH:i * CH + ch.L],
                    initial=init_c, op0=_MUL, op1=_ADD,
                ).then_inc(scC, 1)
                if ch.spool:
                    if POOL_S_IDX[k] == 1:
                        gpsimd.wait_ge(dma_tab, tabv["rSf"])
                        gpsimd.wait_ge(dma_tab, tabv["q2"])
                    if ch.hp:
                        gpsimd.wait_ge(dma_hp, HP_IDX[k] * 32)
                    else:
                        gpsimd.wait_ge(premS, PREM_IDX[k])
                    init_s = 0.0 if ch.first else iq_sb[:, 2 * b + 1:2 * b + 2]
                    gpsimd.tensor_tensor_scan(
                        out=ws_sb[:, j * CH:j * CH + ch.L],
                        data0=rhoS_sb[:, :ch.L],
                        data1=us_sb[:, i * CH:i * CH + ch.L],
                        initial=init_s, op0=_MUL, op1=_ADD,
                    ).then_inc(scSp, 1)
                    if not ch.last:
                        # carry on Pool: both scan ends are local here
                        q = ch.qcol
                        wce = wc_sb[:, j * CH + ch.L - 1:j * CH + ch.L]\
                            .bitcast(_F32)
                        wse = ws_sb[:, j * CH + ch.L - 1:j * CH + ch.L]\
                            .bitcast(_F32)
                        gpsimd.tensor_scalar_mul(
                            out=t0p_sb[:], in0=qc2_sb[:, 2 * q:2 * q + 2],
                            scalar1=wce,
                        )
                        gpsimd.scalar_tensor_tensor(
                            out=iq_sb[:, 2 * b:2 * b + 2],
                            in0=qs2_sb[:, 2 * q:2 * q + 2],
                            scalar=wse, in1=t0p_sb[:], op0=_MUL, op1=_ADD,
                        ).then_inc(carryP, 1)

        @block.tensor
        def _(tensor: bass.BassEngine):
            tensor.wait_ge(dma_tab, tabv["id"])
            for k in range(NCHUNKS):
                ch = CHUNKS[k]
                i2 = k % 2
                j = k % NWSLOT
                tensor.wait_ge(scC, k + 1)
                _s_wait(tensor, k)
                if k >= 2:
                    # WAR: ACT copy of k-2 must have drained this PSUM half
                    tensor.wait_ge(act_y, k - 1)
                nseg = (ch.L + 511) // 512
                mm = None
                for seg in range(nseg):
                    sl = min(512, ch.L - seg * 512)
                    pb = i2 * CH + seg * 512
                    wb = j * CH + seg * 512
                    tensor.matmul(
                        y_ps[:, pb:pb + sl],
                        id_sb[:],
                        wc_sb[:, wb:wb + sl],
                        start=True, stop=False,
                    )
                    mm = tensor.matmul(
                        y_ps[:, pb:pb + sl],
                        id_sb[:],
                        ws_sb[:, wb:wb + sl],
                        start=False, stop=True,
                    )
                mm.then_inc(pe_y, 1)

        @block.scalar
        def _(scalar: bass.BassEngine):
            for k in range(NCHUNKS):
                ch = CHUNKS[k]
                i2 = k % 2
                i4 = k % NYSLOT
                scalar.wait_ge(pe_y, k + 1)
                if k >= NYSLOT:
                    # WAR on y_sb slot: out-DMA of k-NYSLOT must have drained
                    scalar.wait_ge(dma_out, (k - NYSLOT + 1) * 16)
                scalar.copy(
                    out=y_sb[:, i4 * CH:i4 * CH + ch.L],
                    in_=y_ps[:, i2 * CH:i2 * CH + ch.L],
                ).then_inc(act_y, 1)
                # dma_start is a SEQ-level trigger: without this wait it
                # races the still-executing copy on the ACT engine pipe
                scalar.wait_ge(act_y, k + 1)
                scalar.dma_start(
                    out=ys[:, ch.t0:ch.t0 + ch.L],
                    in_=y_sb[:, i4 * CH:i4 * CH + ch.L],
                ).then_inc(dma_out, 16)

    return nc


def _host_tables(decay: np.ndarray, freq: np.ndarray):
    """float64 table construction, cast to fp32/fp16/bf16 at the end."""
    a = np.abs(decay.astype(np.float64))
    f = freq.astype(np.float64)
    damp = np.exp(-a)

    tau = np.arange(CH, dtype=np.float64) + 0.5
    A = f[:, None] * tau[None, :]         # [D, CH]
    c = np.cos(A)
    s = np.sin(A)
    # clamp |cos|, |sin| away from zero so the fp16 ratio tables stay in
    # range (max ratio ~ 1/eps = 125 << fp16 max); the induced kernel error
    # is O(eps^2) at isolated taus.
    eps = 8e-3
    c = np.where(np.abs(c) < eps, np.where(c >= 0, eps, -eps), c)
    s = np.where(np.abs(s) < eps, np.where(s >= 0, eps, -eps), s)
    # weight at tau = -1/2 (the scan-initial position)
    w0c = np.cos(-0.5 * f)
    w0s = np.sin(-0.5 * f)
    w0c = np.where(np.abs(w0c) < eps, eps, w0c)
    w0s = np.where(np.abs(w0s) < eps, np.where(w0s >= 0, eps, -eps), w0s)

    rhoC = np.empty_like(c)
    rhoS = np.empty_like(s)
    rhoC[:, 0] = damp * c[:, 0] / w0c
    rhoS[:, 0] = damp * s[:, 0] / w0s
    rhoC[:, 1:] = damp[:, None] * c[:, 1:] / c[:, :-1]
    rhoS[:, 1:] = damp[:, None] * s[:, 1:] / s[:, :-1]

    # carry across a boundary after a chunk of length L:
    #   g' = e^{+i f L} g, g = C - iS =>
    #   C' = cos(fL) C + sin(fL) S ;  S' = cos(fL) S - sin(fL) C
    #   C_end = Wc_end / c[L-1], S_end = Ws_end / s[L-1]
    #   Winit_C = w0c * C', Winit_S = w0s * S'
    # columns interleaved as [qcc,qsc] (qc2) / [qcs,qss] (qs2) per length
    qc2 = np.empty((len(f), 2 * NQ))
    qs2 = np.empty_like(qc2)
    for qi, L in enumerate(QCOLS):
        rc = np.cos(f * L)
        rs = np.sin(f * L)
        qc2[:, 2 * qi] = w0c * rc / c[:, L - 1]        # qcc
        qc2[:, 2 * qi + 1] = -w0s * rs / c[:, L - 1]   # qsc
        qs2[:, 2 * qi] = w0c * rs / s[:, L - 1]        # qcs
        qs2[:, 2 * qi + 1] = w0s * rc / s[:, L - 1]    # qss

    f32 = np.float32
    return (
        (c * c).astype(ml_dtypes.bfloat16),
        rhoC.astype(np.float16), rhoS.astype(np.float16),
        qc2.astype(f32), qs2.astype(f32),
        c * c, s * s,           # float64 copies for host premult
    )


def kernel(x: np.ndarray, decay: np.ndarray, freq: np.ndarray) -> np.ndarray:
    x = np.asarray(x)
    decay = np.asarray(decay)
    freq = np.asarray(freq)
    assert x.shape == (B, T, D), x.shape
    cc2, rhoC, rhoS, qc2, qs2, cc2_64, ss2_64 = _host_tables(decay, freq)

    # [B,T,D] -> [D, B*T] contiguous, split by core
    xf = np.ascontiguousarray(x.transpose(2, 0, 1).reshape(D, B * T))

    # pack xs (non-hp chunks) and ups (host-premultiplied uc,us pairs)
    bf16 = ml_dtypes.bfloat16
    xs_parts = []
    up_parts = []
    for ch in CHUNKS:
        seg = xf[:, ch.t0:ch.t0 + ch.L]
        if ch.hp:
            up_parts.append((seg * cc2_64[:, :ch.L]).astype(bf16))
            up_parts.append((seg * ss2_64[:, :ch.L]).astype(bf16))
        else:
            xs_parts.append(seg.astype(bf16))
    xs = np.concatenate(xs_parts, axis=1)
    ups = np.concatenate(up_parts, axis=1) if up_parts else \
        np.zeros((D, 0), bf16)

    in_maps = []
    for cidx in range(NCORES):
        lo, hi = cidx * DP, (cidx + 1) * DP
        in_maps.append(
            {
                "xs": np.ascontiguousarray(xs[lo:hi]),
                "ups": np.ascontiguousarray(ups[lo:hi]),
                "cc2": cc2[lo:hi],
                "rhoC": np.ascontiguousarray(rhoC[lo:hi]),
                "rhoS": np.ascontiguousarray(rhoS[lo:hi]),
                "qc2": np.ascontiguousarray(qc2[lo:hi]),
                "qs2": np.ascontiguousarray(qs2[lo:hi]),
                "ident": np.eye(DP, dtype=np.float32),
            }
        )

    nc = _build_nc()
    res = run_bass_kernel_spmd(nc, in_maps, list(range(NCORES)), **_RUN_KW)

    global LAST_RESULT
    LAST_RESULT = res
    y = np.empty((D, B * T), np.float32)
    for cidx in range(NCORES):
        y[cidx * DP:(cidx + 1) * DP] = np.asarray(
            res.results[cidx]["ys"]
        ).astype(np.float32)
    return np.ascontiguousarray(
        y.reshape(D, B, T).transpose(1, 2, 0)
    ).astype(x.dtype)


if __name__ == "__main__":
    rng = np.random.default_rng(0)
    x = rng.standard_normal((B, T, D)).astype(np.float32)
    decay = rng.standard_normal(D).astype(np.float32)
    freq = rng.standard_normal(D).astype(np.float32)
    y = kernel(x, decay, freq)
    print(y.shape, y.dtype, np.abs(y).mean())


# revision 31
# speedup vs baseline: 1.2984x; 1.0622x over previous
"""CausalFFTConv on 8 Trainium2 NeuronCores — bf16-IO scan kernel.

y[b,t,d] = sum_{s<=t} x[b,s,d] * k[t-s,d],  k[t,d] = exp(-|decay_d|*t)*cos(freq_d*t)

W-transformed dual-scan algorithm (see kernel_baseline.py): with chunk-local
half-offset phases A(tau)=f*(tau+1/2), c=cos(A), s=sin(A), the post-multiplied
quantities W_C = c*C, W_S = s*S obey first-order ratio recurrences mapping
onto tensor_tensor_scan; y = W_C + W_S runs on the TensorEngine as identity
matmuls into PSUM, staged out (with an fp32->bf16 cast) by the ACT engine.

Resource budget per core (cost model): DMA ~55us of 360 B/ns traffic
(bf16 x in / bf16 y out / fp16 ratio tables), DVE ~57us (all-bf16 premults
in 2x perf mode + fp32 scans + carries), Pool ~58us (scan_C always +
scan_S for POOL_S chunks at 0.833/0.6 ns/elem), ACT ~31us, PE ~29us.
The schedule exists to keep all of those overlapped:

 * head chunks are host-premultiplied (uc,us uploaded directly) so the
   first scans depend only on two small DMAs, not premult tables;
 * two more mid-stream chunks are host-premultiplied to convert spare DMA
   bandwidth into DVE relief;
 * ss2 is derived on-device as 1-cc2 (exact identity) saving a table DMA;
 * rho tables ship in 2 column pieces so short head chunks start early
   (each DMA->consumer edge costs 900ns of semaphore latency);
 * chunk order interleaves batch pairs so one chain's carry latency hides
   under the other chain's scan (batches reset the scan state);
 * short tail chunks shrink the final scan->PE->ACT->DMA drain.

Sharding: d_model (1024) split 8 ways -> 128 channels per core = the
128 SBUF partitions. Full T per core, batch unrolled on the free axis.
"""

import sys

sys.path.insert(0, "/opt/trn_rl_repo")

from contextlib import ExitStack

import ml_dtypes
import numpy as np

import concourse.bass as bass
import concourse.mybir as mybir
from concourse.bass_utils import run_bass_kernel_spmd

B, T, D = 4, 8192, 1024

_RUN_KW: dict = {}
LAST_RESULT = None

NCORES = 8
DP = D // NCORES        # 128 channels per core == SBUF partitions
CH = 2048               # max chunk length == table extent

# per-batch chunk length schedules (sum = T)
_LENS = {
    0: [512, 1024, 1024, 2048, 2048, 1536],
    1: [512, 1024, 1024, 2048, 2048, 1536],
    2: [2048, 2048, 2048, 1536, 512],
    3: [2048, 2048, 2048, 1536, 512],
}
QCOLS = [512, 1024, 1536, 2048]          # carry-constant column per length

# host-premultiplied chunks (uc/us uploaded, no x, no device premult)
HP_SET = {0, 1, 2, 3, 8, 12, 16}
# chunks whose scan_S runs on Pool (the rest run on DVE)
POOL_S = {7, 13, 17}


def _mk_order():
    out = []
    for bp in (0, 2):
        for c in range(len(_LENS[bp])):
            out.append((bp, c))
            out.append((bp + 1, c))
    return out


_ORDER = _mk_order()
NCHUNKS = len(_ORDER)       # 22


class _Chunk:
    __slots__ = ("k", "b", "c", "t0", "L", "first", "last", "qcol", "hp",
                 "spool")

    def __init__(self, k, b, c, t0, L, first, last, qcol, hp, spool):
        self.k, self.b, self.c, self.t0, self.L = k, b, c, t0, L
        self.first, self.last, self.qcol = first, last, qcol
        self.hp, self.spool = hp, spool


def _mk_chunks():
    offs = {b: b * T for b in range(4)}
    chunks = []
    for k, (b, c) in enumerate(_ORDER):
        L = _LENS[b][c]
        t0 = offs[b]
        offs[b] += L
        chunks.append(_Chunk(
            k, b, c, t0, L,
            first=(c == 0), last=(c == len(_LENS[b]) - 1),
            qcol=QCOLS.index(L), hp=(k in HP_SET), spool=(k in POOL_S),
        ))
    return chunks


CHUNKS = _mk_chunks()
FIRST_NONHP = min(ch.k for ch in CHUNKS if not ch.hp)
# first chunk needing full-width (>1024) tables on each scan path
FIRST_WIDE = min(ch.k for ch in CHUNKS if ch.L > 1024)
FIRST_WIDE_SD = min(ch.k for ch in CHUNKS if ch.L > 1024 and not ch.spool)
# cumulative counters (value of the sem once chunk k's item is done)
PREM_IDX = np.cumsum([0 if ch.hp else 1 for ch in CHUNKS]).tolist()
HP_IDX = np.cumsum([1 if ch.hp else 0 for ch in CHUNKS]).tolist()
POOL_S_IDX = np.cumsum([1 if ch.spool else 0 for ch in CHUNKS]).tolist()
DVE_S_IDX = np.cumsum([0 if ch.spool else 1 for ch in CHUNKS]).tolist()
# carries run on the engine that produced the chunk's scan_S: Pool for
# POOL_S chunks, DVE otherwise.  Separate counters per engine.
CARRY_D_IDX = np.cumsum(
    [1 if (not ch.last and not ch.spool) else 0 for ch in CHUNKS]).tolist()
CARRY_P_IDX = np.cumsum(
    [1 if (not ch.last and ch.spool) else 0 for ch in CHUNKS]).tolist()

_F32 = mybir.dt.float32
_F32R = mybir.dt.float32r
_F16 = mybir.dt.float16
_BF16 = mybir.dt.bfloat16
_MUL = mybir.AluOpType.mult
_ADD = mybir.AluOpType.add

NXSLOT = 6      # x / uc / us chunk slots
NWSLOT = 4      # wc / ws scan-output slots
NYSLOT = 4      # y staging slots
PREM_AHEAD = 2  # premult emission lookahead (chunks)
NQ = len(QCOLS)


def _build_nc():
    nc = bass.Bass()
    xs_len = sum(ch.L for ch in CHUNKS if not ch.hp)
    up_len = sum(2 * ch.L for ch in CHUNKS if ch.hp)
    xs = nc.declare_dram_parameter("xs", [DP, xs_len], _BF16, isOutput=False)
    ups = nc.declare_dram_parameter("ups", [DP, up_len], _BF16, isOutput=False)
    cc2 = nc.declare_dram_parameter("cc2", [DP, CH], _BF16, isOutput=False)
    rhoC = nc.declare_dram_parameter("rhoC", [DP, CH], _F16, isOutput=False)
    rhoS = nc.declare_dram_parameter("rhoS", [DP, CH], _F16, isOutput=False)
    # fused carry constants, one column pair per chunk length:
    #   Winit_C = qc2[:,2q]*WcEnd + qs2[:,2q]*WsEnd
    #   Winit_S = qc2[:,2q+1]*WcEnd + qs2[:,2q+1]*WsEnd
    qc2 = nc.declare_dram_parameter("qc2", [DP, 2 * NQ], _F32, isOutput=False)
    qs2 = nc.declare_dram_parameter("qs2", [DP, 2 * NQ], _F32, isOutput=False)
    ident = nc.declare_dram_parameter("ident", [DP, DP], _F32R, isOutput=False)
    ys = nc.declare_dram_parameter("ys", [DP, B * T], _BF16, isOutput=True)

    # per-chunk offsets into xs / ups
    xoff = {}
    uoff = {}
    xo = uo = 0
    for ch in CHUNKS:
        if ch.hp:
            uoff[ch.k] = uo
            uo += 2 * ch.L
        else:
            xoff[ch.k] = xo
            xo += ch.L

    with ExitStack() as ctx:
        ent = ctx.enter_context
        cc2_sb = ent(nc.sbuf_tensor([DP, CH], _BF16))
        ss2_sb = ent(nc.sbuf_tensor([DP, CH], _BF16))
        rhoC_sb = ent(nc.sbuf_tensor([DP, CH], _F16))
        rhoS_sb = ent(nc.sbuf_tensor([DP, CH], _F16))
        qc2_sb = ent(nc.sbuf_tensor([DP, 2 * NQ], _F32))
        qs2_sb = ent(nc.sbuf_tensor([DP, 2 * NQ], _F32))
        xt_sb = ent(nc.sbuf_tensor([DP, NXSLOT * CH], _BF16))
        uc_sb = ent(nc.sbuf_tensor([DP, NXSLOT * CH], _BF16))
        us_sb = ent(nc.sbuf_tensor([DP, NXSLOT * CH], _BF16))
        id_sb = ent(nc.sbuf_tensor([DP, DP], _F32R))
        y_sb = ent(nc.sbuf_tensor([DP, NYSLOT * CH], _BF16))
        wc_sb = ent(nc.sbuf_tensor([DP, NWSLOT * CH], _F32R))
        ws_sb = ent(nc.sbuf_tensor([DP, NWSLOT * CH], _F32R))
        iq_sb = ent(nc.sbuf_tensor([DP, 2 * 4], _F32))   # per-batch inits
        t0_sb = ent(nc.sbuf_tensor([DP, 2], _F32))       # DVE carry scratch
        t0p_sb = ent(nc.sbuf_tensor([DP, 2], _F32))      # Pool carry scratch
        y_ps = ent(nc.psum_tensor([DP, 2 * CH], _F32))
        dma_in = ent(nc.semaphore("dma_in"))
        dma_hp = ent(nc.semaphore("dma_hp"))
        dma_tab = ent(nc.semaphore("dma_tab"))
        dma_out = ent(nc.semaphore("dma_out"))
        premC = ent(nc.semaphore("premC"))
        premS = ent(nc.semaphore("premS"))
        scC = ent(nc.semaphore("scC"))
        scSd = ent(nc.semaphore("scSd"))
        scSp = ent(nc.semaphore("scSp"))
        carryD = ent(nc.semaphore("carryD"))
        carryP = ent(nc.semaphore("carryP"))
        pe_y = ent(nc.semaphore("pe_y"))
        act_y = ent(nc.semaphore("act_y"))
        block = ent(nc.Block(no_gpsimd_drain=True))

        # table DMA sem values, recorded while emitting the sync program
        tabv = {}

        def _s_wait(eng, k):
            """wait until chunk k's scan_S is complete"""
            if CHUNKS[k].spool:
                eng.wait_ge(scSp, POOL_S_IDX[k])
            else:
                eng.wait_ge(scSd, DVE_S_IDX[k])

        def _carry_wait(eng, j, own):
            """wait until chunk j's carry is complete (own = this engine
            runs that carry itself, so program order covers it)"""
            if j < 0 or CHUNKS[j].last:
                return
            if CHUNKS[j].spool:
                if own != "pool":
                    eng.wait_ge(carryP, CARRY_P_IDX[j])
            else:
                if own != "dve":
                    eng.wait_ge(carryD, CARRY_D_IDX[j])

        @block.sync
        def _(sync: bass.BassEngine):
            ntab = [0]

            def tab(name, out, in_):
                sync.dma_start(out=out, in_=in_).then_inc(dma_tab, 16)
                ntab[0] += 16
                tabv[name] = ntab[0]

            def xdma(k):
                ch = CHUNKS[k]
                i = k % NXSLOT
                if not ch.hp:
                    j = k - NXSLOT
                    if j >= 0:
                        # WAR: slot consumers of chunk j must be done.  The
                        # xt slot is read by premults; uc/us by the scans.
                        if CHUNKS[j].hp:
                            sync.wait_ge(scC, j + 1)
                            _s_wait(sync, j)
                        else:
                            sync.wait_ge(premC, PREM_IDX[j])
                            sync.wait_ge(premS, PREM_IDX[j])
                    sync.dma_start(
                        out=xt_sb[:, i * CH:i * CH + ch.L],
                        in_=xs[:, xoff[k]:xoff[k] + ch.L],
                    ).then_inc(dma_in, 16)
                else:
                    j = k - NXSLOT
                    if j >= 0:
                        # WAR on uc/us slots: scans of j must be done
                        sync.wait_ge(scC, j + 1)
                        _s_wait(sync, j)
                    uo = uoff[k]
                    sync.dma_start(
                        out=uc_sb[:, i * CH:i * CH + ch.L],
                        in_=ups[:, uo:uo + ch.L],
                    ).then_inc(dma_hp, 16)
                    sync.dma_start(
                        out=us_sb[:, i * CH:i * CH + ch.L],
                        in_=ups[:, uo + ch.L:uo + 2 * ch.L],
                    ).then_inc(dma_hp, 16)

            # head: hp chunks 0..3 interleaved with table pieces
            xdma(0)
            tab("rCp", rhoC_sb[:, :1024], rhoC[:, :1024])
            tab("rSp", rhoS_sb[:, :1024], rhoS[:, :1024])
            tab("id", id_sb[:], ident[:])
            tab("q", qc2_sb[:], qc2[:])
            tab("q2", qs2_sb[:], qs2[:])
            xdma(1)
            xdma(2)
            xdma(3)
            tab("cc2", cc2_sb[:], cc2[:])
            xdma(4)
            xdma(5)
            tab("rCf", rhoC_sb[:, 1024:], rhoC[:, 1024:])
            tab("rSf", rhoS_sb[:, 1024:], rhoS[:, 1024:])
            # output DMAs are issued from here (ACT's own dma_start would
            # serialize behind its copy-completion wait, stalling the copy
            # pipeline); interleaved with the input DMAs at a lag so
            # neither side's WAR waits block the other's issuance.
            def ydma(k):
                ch = CHUNKS[k]
                i4 = k % NYSLOT
                sync.wait_ge(act_y, k + 1)
                sync.dma_start(
                    out=ys[:, ch.t0:ch.t0 + ch.L],
                    in_=y_sb[:, i4 * CH:i4 * CH + ch.L],
                ).then_inc(dma_out, 16)

            for k in range(6, NCHUNKS):
                xdma(k)
                if k >= 10:
                    ydma(k - 10)
            for k in range(NCHUNKS - 10, NCHUNKS):
                ydma(k)
            sync.wait_ge(dma_out, NCHUNKS * 16)

        @block.vector
        def _(vector: bass.BassEngine):
            def prem(k):
                if k >= NCHUNKS or CHUNKS[k].hp:
                    return
                ch = CHUNKS[k]
                i = k % NXSLOT
                xt = xt_sb[:, i * CH:i * CH + ch.L]
                vector.wait_ge(dma_in, PREM_IDX[k] * 16)
                if k == FIRST_NONHP:
                    vector.wait_ge(dma_tab, tabv["cc2"])
                j = k - NXSLOT
                if j >= 0:
                    # WAR on uc slot: scan_C of j (on Pool) read it
                    vector.wait_ge(scC, j + 1)
                vector.tensor_tensor(
                    out=uc_sb[:, i * CH:i * CH + ch.L], in0=xt,
                    in1=cc2_sb[:, :ch.L], op=_MUL,
                ).then_inc(premC, 1)
                if k == FIRST_NONHP:
                    # ss2 = 1 - cc2 (exact identity), bf16 4x tensor_scalar
                    vector.tensor_scalar(
                        out=ss2_sb[:], in0=cc2_sb[:], scalar1=-1.0,
                        scalar2=1.0, op0=_MUL, op1=_ADD,
                    )
                if j >= 0 and CHUNKS[j].spool:
                    # WAR on us slot (DVE-run scan_S is ordered by our queue)
                    vector.wait_ge(scSp, POOL_S_IDX[j])
                vector.tensor_tensor(
                    out=us_sb[:, i * CH:i * CH + ch.L], in0=xt,
                    in1=ss2_sb[:, :ch.L], op=_MUL,
                ).then_inc(premS, 1)

            for kk in range(PREM_AHEAD):
                prem(kk)
            for k in range(NCHUNKS):
                ch = CHUNKS[k]
                b = ch.b
                j = k % NWSLOT
                i = k % NXSLOT

                if not ch.spool:
                    # scan_S on DVE
                    if k == 0:
                        vector.wait_ge(dma_tab, tabv["rSp"])
                    elif k == FIRST_WIDE_SD:
                        vector.wait_ge(dma_tab, tabv["rSf"])
                    if ch.hp:
                        vector.wait_ge(dma_hp, HP_IDX[k] * 32)
                    _carry_wait(vector, k - 2, "dve")          # chain init
                    _carry_wait(vector, k - 3, "dve")          # WAR ws end
                    if k >= NWSLOT:
                        vector.wait_ge(pe_y, k - NWSLOT + 1)   # WAR ws slot
                    init_s = 0.0 if ch.first else iq_sb[:, 2 * b + 1:2 * b + 2]
                    vector.tensor_tensor_scan(
                        out=ws_sb[:, j * CH:j * CH + ch.L],
                        data0=rhoS_sb[:, :ch.L],
                        data1=us_sb[:, i * CH:i * CH + ch.L],
                        initial=init_s, op0=_MUL, op1=_ADD,
                    ).then_inc(scSd, 1)

                if not ch.last and not ch.spool:
                    # fused carry for chunk (b, c+1):
                    #   t0 = [qcc,qsc]*WcEnd ; iq[2b:2b+2] = [qcs,qss]*WsEnd+t0
                    if k == 0:
                        vector.wait_ge(dma_tab, tabv["q2"])
                    vector.wait_ge(scC, k + 1)
                    q = ch.qcol
                    wce = wc_sb[:, j * CH + ch.L - 1:j * CH + ch.L].bitcast(_F32)
                    wse = ws_sb[:, j * CH + ch.L - 1:j * CH + ch.L].bitcast(_F32)
                    vector.tensor_scalar_mul(
                        out=t0_sb[:], in0=qc2_sb[:, 2 * q:2 * q + 2],
                        scalar1=wce,
                    )
                    vector.scalar_tensor_tensor(
                        out=iq_sb[:, 2 * b:2 * b + 2],
                        in0=qs2_sb[:, 2 * q:2 * q + 2],
                        scalar=wse, in1=t0_sb[:], op0=_MUL, op1=_ADD,
                    ).then_inc(carryD, 1)

                prem(k + PREM_AHEAD)

        @block.gpsimd
        def _(gpsimd: bass.BassEngine):
            for k in range(NCHUNKS):
                ch = CHUNKS[k]
                b = ch.b
                j = k % NWSLOT
                i = k % NXSLOT
                if k == 0:
                    gpsimd.wait_ge(dma_tab, tabv["rCp"])
                elif k == FIRST_WIDE:
                    gpsimd.wait_ge(dma_tab, tabv["rCf"])
                if ch.hp:
                    gpsimd.wait_ge(dma_hp, HP_IDX[k] * 32 - 16)
                else:
                    gpsimd.wait_ge(premC, PREM_IDX[k])
                _carry_wait(gpsimd, k - 2, "pool")          # chain init
                _carry_wait(gpsimd, k - 3, "pool")          # WAR wc end
                if k >= NWSLOT:
                    gpsimd.wait_ge(pe_y, k - NWSLOT + 1)    # WAR wc slot
                init_c = 0.0 if ch.first else iq_sb[:, 2 * b:2 * b + 1]
                gpsimd.tensor_tensor_scan(
                    out=wc_sb[:, j * CH:j * CH + ch.L],
                    data0=rhoC_sb[:, :ch.L],
                    data1=uc_sb[:, i * CH:i * CH + ch.L],
                    initial=init_c, op0=_MUL, op1=_ADD,
                ).then_inc(scC, 1)
                if ch.spool:
                    if POOL_S_IDX[k] == 1:
                        gpsimd.wait_ge(dma_tab, tabv["rSf"])
                        gpsimd.wait_ge(dma_tab, tabv["q2"])
                    if ch.hp:
                        gpsimd.wait_ge(dma_hp, HP_IDX[k] * 32)
                    else:
                        gpsimd.wait_ge(premS, PREM_IDX[k])
                    init_s = 0.0 if ch.first else iq_sb[:, 2 * b + 1:2 * b + 2]
                    gpsimd.tensor_tensor_scan(
                        out=ws_sb[:, j * CH:j * CH + ch.L],
                        data0=rhoS_sb[:, :ch.L],
                        data1=us_sb[:, i * CH:i * CH + ch.L],
                        initial=init_s, op0=_MUL, op1=_ADD,
                    ).then_inc(scSp, 1)
                    if not ch.last:
                        # carry on Pool: both scan ends are local here
                        q = ch.qcol
                        wce = wc_sb[:, j * CH + ch.L - 1:j * CH + ch.L]\
                            .bitcast(_F32)
                        wse = ws_sb[:, j * CH + ch.L - 1:j * CH + ch.L]\
                            .bitcast(_F32)
                        gpsimd.tensor_scalar_mul(
                            out=t0p_sb[:], in0=qc2_sb[:, 2 * q:2 * q + 2],
                            scalar1=wce,
                        )
                        gpsimd.scalar_tensor_tensor(
                            out=iq_sb[:, 2 * b:2 * b + 2],
                            in0=qs2_sb[:, 2 * q:2 * q + 2],
                            scalar=wse, in1=t0p_sb[:], op0=_MUL, op1=_ADD,
                        ).then_inc(carryP, 1)

        @block.tensor
        def _(tensor: bass.BassEngine):
            tensor.wait_ge(dma_tab, tabv["id"])
            for k in range(NCHUNKS):
                ch = CHUNKS[k]
                i2 = k % 2
                j = k % NWSLOT
                tensor.wait_ge(scC, k + 1)
                _s_wait(tensor, k)
                if k >= 2:
                    # WAR: ACT copy of k-2 must have drained this PSUM half
                    tensor.wait_ge(act_y, k - 1)
                nseg = (ch.L + 511) // 512
                mm = None
                for seg in range(nseg):
                    sl = min(512, ch.L - seg * 512)
                    pb = i2 * CH + seg * 512
                    wb = j * CH + seg * 512
                    tensor.matmul(
                        y_ps[:, pb:pb + sl],
                        id_sb[:],
                        wc_sb[:, wb:wb + sl],
                        start=True, stop=False,
                    )
                    mm = tensor.matmul(
                        y_ps[:, pb:pb + sl],
                        id_sb[:],
                        ws_sb[:, wb:wb + sl],
                        start=False, stop=True,
                    )
                mm.then_inc(pe_y, 1)

        @block.scalar
        def _(scalar: bass.BassEngine):
            for k in range(NCHUNKS):
                ch = CHUNKS[k]
                i2 = k % 2
                i4 = k % NYSLOT
                scalar.wait_ge(pe_y, k + 1)
                if k >= NYSLOT:
                    # WAR on y_sb slot: out-DMA of k-NYSLOT must have drained
                    scalar.wait_ge(dma_out, (k - NYSLOT + 1) * 16)
                scalar.copy(
                    out=y_sb[:, i4 * CH:i4 * CH + ch.L],
                    in_=y_ps[:, i2 * CH:i2 * CH + ch.L],
                ).then_inc(act_y, 1)

    return nc


def _host_tables(decay: np.ndarray, freq: np.ndarray):
    """float64 table construction, cast to fp32/fp16/bf16 at the end."""
    a = np.abs(decay.astype(np.float64))
    f = freq.astype(np.float64)
    damp = np.exp(-a)

    tau = np.arange(CH, dtype=np.float64) + 0.5
    A = f[:, None] * tau[None, :]         # [D, CH]
    c = np.cos(A)
    s = np.sin(A)
    # clamp |cos|, |sin| away from zero so the fp16 ratio tables stay in
    # range (max ratio ~ 1/eps = 125 << fp16 max); the induced kernel error
    # is O(eps^2) at isolated taus.
    eps = 8e-3
    c = np.where(np.abs(c) < eps, np.where(c >= 0, eps, -eps), c)
    s = np.where(np.abs(s) < eps, np.where(s >= 0, eps, -eps), s)
    # weight at tau = -1/2 (the scan-initial position)
    w0c = np.cos(-0.5 * f)
    w0s = np.sin(-0.5 * f)
    w0c = np.where(np.abs(w0c) < eps, eps, w0c)
    w0s = np.where(np.abs(w0s) < eps, np.where(w0s >= 0, eps, -eps), w0s)

    rhoC = np.empty_like(c)
    rhoS = np.empty_like(s)
    rhoC[:, 0] = damp * c[:, 0] / w0c
    rhoS[:, 0] = damp * s[:, 0] / w0s
    rhoC[:, 1:] = damp[:, None] * c[:, 1:] / c[:, :-1]
    rhoS[:, 1:] = damp[:, None] * s[:, 1:] / s[:, :-1]

    # carry across a boundary after a chunk of length L:
    #   g' = e^{+i f L} g, g = C - iS =>
    #   C' = cos(fL) C + sin(fL) S ;  S' = cos(fL) S - sin(fL) C
    #   C_end = Wc_end / c[L-1], S_end = Ws_end / s[L-1]
    #   Winit_C = w0c * C', Winit_S = w0s * S'
    # columns interleaved as [qcc,qsc] (qc2) / [qcs,qss] (qs2) per length
    qc2 = np.empty((len(f), 2 * NQ))
    qs2 = np.empty_like(qc2)
    for qi, L in enumerate(QCOLS):
        rc = np.cos(f * L)
        rs = np.sin(f * L)
        qc2[:, 2 * qi] = w0c * rc / c[:, L - 1]        # qcc
        qc2[:, 2 * qi + 1] = -w0s * rs / c[:, L - 1]   # qsc
        qs2[:, 2 * qi] = w0c * rs / s[:, L - 1]        # qcs
        qs2[:, 2 * qi + 1] = w0s * rc / s[:, L - 1]    # qss

    f32 = np.float32
    return (
        (c * c).astype(ml_dtypes.bfloat16),
        rhoC.astype(np.float16), rhoS.astype(np.float16),
        qc2.astype(f32), qs2.astype(f32),
        c * c, s * s,           # float64 copies for host premult
    )


def kernel(x: np.ndarray, decay: np.ndarray, freq: np.ndarray) -> np.ndarray:
    x = np.asarray(x)
    decay = np.asarray(decay)
    freq = np.asarray(freq)
    assert x.shape == (B, T, D), x.shape
    cc2, rhoC, rhoS, qc2, qs2, cc2_64, ss2_64 = _host_tables(decay, freq)

    # [B,T,D] -> [D, B*T] contiguous, split by core
    xf = np.ascontiguousarray(x.transpose(2, 0, 1).reshape(D, B * T))

    # pack xs (non-hp chunks) and ups (host-premultiplied uc,us pairs)
    bf16 = ml_dtypes.bfloat16
    xs_parts = []
    up_parts = []
    for ch in CHUNKS:
        seg = xf[:, ch.t0:ch.t0 + ch.L]
        if ch.hp:
            up_parts.append((seg * cc2_64[:, :ch.L]).astype(bf16))
            up_parts.append((seg * ss2_64[:, :ch.L]).astype(bf16))
        else:
            xs_parts.append(seg.astype(bf16))
    xs = np.concatenate(xs_parts, axis=1)
    ups = np.concatenate(up_parts, axis=1) if up_parts else \
        np.zeros((D, 0), bf16)

    in_maps = []
    for cidx in range(NCORES):
        lo, hi = cidx * DP, (cidx + 1) * DP
        in_maps.append(
            {
                "xs": np.ascontiguousarray(xs[lo:hi]),
                "ups": np.ascontiguousarray(ups[lo:hi]),
                "cc2": cc2[lo:hi],
                "rhoC": np.ascontiguousarray(rhoC[lo:hi]),
                "rhoS": np.ascontiguousarray(rhoS[lo:hi]),
                "qc2": np.ascontiguousarray(qc2[lo:hi]),
                "qs2": np.ascontiguousarray(qs2[lo:hi]),
                "ident": np.eye(DP, dtype=np.float32),
            }
        )

    nc = _build_nc()
    res = run_bass_kernel_spmd(nc, in_maps, list(range(NCORES)), **_RUN_KW)

    global LAST_RESULT
    LAST_RESULT = res
    y = np.empty((D, B * T), np.float32)
    for cidx in range(NCORES):
        y[cidx * DP:(cidx + 1) * DP] = np.asarray(
            res.results[cidx]["ys"]
        ).astype(np.float32)
    return np.ascontiguousarray(
        y.reshape(D, B, T).transpose(1, 2, 0)
    ).astype(x.dtype)


if __name__ == "__main__":
    rng = np.random.default_rng(0)
    x = rng.standard_normal((B, T, D)).astype(np.float32)
    decay = rng.standard_normal(D).astype(np.float32)
    freq = rng.standard_normal(D).astype(np.float32)
    y = kernel(x, decay, freq)
    print(y.shape, y.dtype, np.abs(y).mean())
